# revision 1
# baseline (speedup 1.0000x reference)
"""Causal self-attention (B=2, S=2048, dim=1024, 16 heads, RoPE) on 8 trn2 cores.

Sharding: batch x head-group. Core c handles batch c//4 and heads [4*(c%4), 4*(c%4)+4).
QKV is column-parallel (each core computes Q/K/V only for its 4 heads), attention is
embarrassingly parallel per (batch, head), output projection is row-parallel
(each core computes a partial [S, dim] product over its heads' 256 attn dims);
the host sums the 4 partials per batch (pure unshard of the sum-sharded output).

Device pipeline per core (all matmuls bf16, accumulation fp32 in PSUM):
  A) QKV: lhsT = x^T tile (host-pretransposed bf16), rhs = w_qkv column slice.
  B) RoPE on Q,K in token-major layout (DVE, fp32 tables), cast to bf16,
     PE-transpose to Q^T/K^T [64*2h, S] for the attention matmuls.
  C) Per (head, q-chunk of 512): scores^T = K^T_tile.T @ Q^T chunk -> PSUM,
     exp via ScalarE (scale=1/8 folded in, no max subtraction -- logits are
     O(6) here so exp is safe in fp32), causal handled by skipping fully
     masked tiles, restricting matmul/exp columns, and a gpsimd affine_select
     on the one triangular 128x128 block per k-tile.  P^T lands in SBUF bf16.
     AV: lhsT = V k-tile augmented with a ones column -> out [65, 512] where
     row 64 is the softmax denominator; DVE rescales rows 0..63 by 1/denom.
  D) proj: lhsT = packed O^T [128, t], rhs = w_proj row-slice; bias/4 added
     during the PSUM->SBUF copyback; DMA partial out.
"""

import sys

sys.path.insert(0, "/opt/trn_rl_repo")

import numpy as np

B = 2
S = 2048
DM = 1024
NH = 16
HD = 64
NCORES = 8
HPC = 4          # heads per core
TT = S // 128    # 16 token tiles
QC = 4           # q-chunks of 512
MAX_WAVELENGTH = 10000.0

_cache = {}


def _build_nc(phases=7, reps=1, av_inter=False, s_bufs=2, o_bufs=1, qk_bufs=2,
              share_v=False, with_bias=True):
    _vtag = "tr" if share_v else "v" 
    import concourse.bass as bass
    import concourse.tile as tile
    import concourse.mybir as mybir
    from concourse import bacc
    from concourse.masks import make_identity

    F32 = mybir.dt.float32
    BF16 = mybir.dt.bfloat16
    Exp = mybir.ActivationFunctionType.Exp

    nc = bacc.Bacc()

    xT = nc.dram_tensor("xT", [DM, S], BF16, kind="ExternalInput")
    wqk = nc.dram_tensor("wqk", [DM, 512], BF16, kind="ExternalInput")
    wv = nc.dram_tensor("wv", [DM, 256], BF16, kind="ExternalInput")
    wp = nc.dram_tensor("wp", [256, DM], BF16, kind="ExternalInput")
    bias4 = nc.dram_tensor("bias4", [1, DM], F32, kind="ExternalInput")
    cos_t = nc.dram_tensor("cos_t", [S, HD], F32, kind="ExternalInput")
    sin_t = nc.dram_tensor("sin_t", [S, HD], F32, kind="ExternalInput")
    out = nc.dram_tensor("out_partial", [S, DM], F32, kind="ExternalOutput")

    with tile.TileContext(nc) as tc:
        with tc.tile_pool(name="persist", bufs=1) as persist, \
             tc.tile_pool(name="ropep", bufs=2) as ropep, \
             tc.tile_pool(name="pTp", bufs=2) as pTp, \
             tc.tile_pool(name="smallp", bufs=3) as smallp, \
             tc.tile_pool(name="outp", bufs=4) as outp, \
             tc.tile_pool(name="psB", bufs=qk_bufs, space="PSUM") as psB, \
             tc.tile_pool(name="psT", bufs=1, space="PSUM") as psT, \
             tc.tile_pool(name="psS", bufs=s_bufs, space="PSUM") as psS, \
             tc.tile_pool(name="psO", bufs=o_bufs, space="PSUM") as psO:
            ident = persist.tile([128, 128], BF16)
            make_identity(nc, ident)

            for _rep in range(reps):
              # --- constant loads -------------------------------------------------
              xT_sb = persist.tile([128, 8, S], BF16)
              xTr = xT.rearrange("(mc p) t -> p mc t", p=128)
              for mm in range(8):
                  nc.sync.dma_start(xT_sb[:, mm, :], xTr[:, mm, :])
              wqk_sb = persist.tile([128, 8, 512], BF16)
              nc.sync.dma_start(wqk_sb, wqk.rearrange("(mc p) c -> p mc c", p=128))
              wv_sb = persist.tile([128, 8, 256], BF16)
              nc.sync.dma_start(wv_sb, wv.rearrange("(mc p) c -> p mc c", p=128))
              cos_sb = persist.tile([128, TT, HD], F32)
              nc.sync.dma_start(cos_sb, cos_t.rearrange("(tt p) d -> p tt d", p=128))
              sin_sb = persist.tile([128, TT, HD], F32)
              nc.sync.dma_start(sin_sb, sin_t.rearrange("(tt p) d -> p tt d", p=128))
              bias_row = persist.tile([1, DM], F32)
              nc.sync.dma_start(bias_row, bias4[:, :])
              if with_bias:
                  bias_b = persist.tile([128, DM], F32)
                  nc.gpsimd.partition_broadcast(bias_b, bias_row, channels=128)
              wp_sb = persist.tile([128, 2, DM], BF16)
              nc.sync.dma_start(wp_sb, wp.rearrange("(kc p) n -> p kc n", p=128))

              # PE warm-up: keep TensorE busy during the initial DMAs so the
              # HAM clock gate is at 2.4 GHz when real matmuls arrive.
              warm = psT.tile([128, 128], BF16, tag="tr", name="warm")
              for _w in range(24):
                  nc.tensor.transpose(warm, ident, ident)

              # V in token-major with a ones column per head, one tile per
              # token-tile so attention only depends on the tiles it reads
              v_tiles = {}
              for tt in range(TT):
                  v_tiles[tt] = persist.tile([128, HPC, 65], BF16, tag=f"v_{tt}", name=f"v_{tt}")
                  nc.vector.memset(v_tiles[tt][:, :, 64:65], 1.0)
              # Q^T/K^T (roped, bf16), split per (cc, q-chunk of 512);
              # cc: 0=Qh01 1=Qh23 2=Kh01 3=Kh23
              qkT = {}
              for cc in range(4):
                  for qi in range(QC):
                      qkT[(cc, qi)] = persist.tile([128, 512], BF16,
                                                   tag=f"qkT_{cc}_{qi}",
                                                   name=f"qkT_{cc}_{qi}")
              # packed O^T for proj lhsT, split per q-chunk (even heads written
              # directly by DVE, odd heads bounced via cross-partition DMA)
              oT_tiles = {}
              for qi in range(QC):
                  oT_tiles[qi] = persist.tile([128, 2, 512], BF16, tag=f"oT_{qi}", name=f"oT_{qi}")

              def emit_qkv(tt):
                  ts = slice(tt * 128, (tt + 1) * 128)
                  psqk = psB.tile([128, 512], F32, tag="qk")
                  for mm in range(8):
                      nc.tensor.matmul(psqk, xT_sb[:, mm, ts], wqk_sb[:, mm, :],
                                       start=(mm == 0), stop=(mm == 7))
                  psv_full = psB.tile([128, 512], F32, tag="qk", name="psv")
                  psv = psv_full[:, 0:256]
                  for mm in range(8):
                      nc.tensor.matmul(psv, xT_sb[:, mm, ts], wv_sb[:, mm, :],
                                       start=(mm == 0), stop=(mm == 7))

                  # RoPE on the 8 (4Q + 4K) 64-wide head blocks of psqk
                  tmp = ropep.tile([128, 512], F32, tag="tmp")
                  tmp2 = ropep.tile([128, 512], F32, tag="tmp2")
                  qkro = ropep.tile([128, 512], BF16, tag="qkro")
                  pv = psqk.rearrange("p (b h s) -> p b h s", b=8, h=2, s=32)
                  tv = tmp.rearrange("p (b h s) -> p b h s", b=8, h=2, s=32)
                  s0 = sin_sb[:, tt, None, 0:32].to_broadcast([128, 8, 32])
                  s1 = sin_sb[:, tt, None, 32:64].to_broadcast([128, 8, 32])
                  cb = cos_sb[:, tt, None, :].to_broadcast([128, 8, HD])
                  nc.vector.tensor_tensor(tv[:, :, 0, :], pv[:, :, 1, :], s0,
                                          mybir.AluOpType.mult)
                  nc.vector.tensor_tensor(tv[:, :, 1, :], pv[:, :, 0, :], s1,
                                          mybir.AluOpType.mult)
                  tv2 = tmp2.rearrange("p (b d) -> p b d", b=8)
                  nc.vector.tensor_tensor(tv2, pv.rearrange("p b h s -> p b (h s)"),
                                          cb, mybir.AluOpType.mult)
                  nc.vector.tensor_tensor(qkro, tmp2, tmp, mybir.AluOpType.add)

                  # V copyback (cast to bf16) into the ones-augmented layout
                  nc.scalar.copy(out=v_tiles[tt][:, :, 0:64],
                                 in_=psv.rearrange("p (h d) -> p h d", h=HPC))

                  # transpose roped QK into qkT
                  cslice = slice((tt % 4) * 128, (tt % 4 + 1) * 128)
                  for cc in range(4):
                      ptr = psT.tile([128, 128], BF16, tag="tr")
                      nc.tensor.transpose(ptr, qkro[:, cc * 128:(cc + 1) * 128],
                                          ident)
                      if cc % 2 == 0:
                          nc.vector.tensor_copy(out=qkT[(cc, tt // 4)][:, cslice],
                                                in_=ptr)
                      else:
                          nc.scalar.copy(out=qkT[(cc, tt // 4)][:, cslice], in_=ptr)

              def emit_attn(qc, h):
                  n_kt = 4 * (qc + 1)
                  pbase = (h % 2) * 64
                  qT = qkT[(h // 2, qc)][pbase:pbase + 64, :]
                  pT = pTp.tile([128, TT, 512], BF16, tag="pT")
                  pso = psO.tile([65, 512], F32, tag="o")
                  # scores come in pairs of k-tiles sharing one 2-bank PSUM
                  # buffer so non-diagonal pairs need only one exp instruction
                  for kp in range(n_kt // 2):
                      ps2 = psS.tile([128, 2, 512], F32, tag="s")
                      for ki in range(2):
                          kt = kp * 2 + ki
                          j = kt - 4 * qc  # >=0 on diagonal-crossing tiles
                          cs = max(0, j * 128)
                          kT = qkT[(2 + h // 2, kt // 4)][pbase:pbase + 64,
                                                          (kt % 4) * 128:(kt % 4 + 1) * 128]
                          nc.tensor.matmul(
                              ps2[:, ki, cs:512],
                              kT,
                              qT[:, cs:512],
                              start=True, stop=True)
                      if kp * 2 < 4 * qc:  # both tiles full: single big exp
                          nc.scalar.activation(
                              out=pT[:, kp * 2:kp * 2 + 2, :],
                              in_=ps2,
                              func=Exp, scale=0.125)
                      else:
                          for ki in range(2):
                              kt = kp * 2 + ki
                              j = kt - 4 * qc
                              cs = j * 128
                              nc.scalar.activation(out=pT[:, kt, cs:512],
                                                   in_=ps2[:, ki, cs:512],
                                                   func=Exp, scale=0.125)
                      for ki in range(2):
                          kt = kp * 2 + ki
                          j = kt - 4 * qc
                          if j >= 0:
                              blk = slice(j * 128, (j + 1) * 128)
                              nc.gpsimd.affine_select(
                                  out=pT[:, kt, blk], in_=pT[:, kt, blk],
                                  pattern=[[1, 128]], channel_multiplier=-1,
                                  base=0, compare_op=mybir.AluOpType.is_ge,
                                  fill=0.0)
                      if av_inter:
                          nc.tensor.matmul(pso[:, cs:512],
                                           v_tiles[kt][:, h, :],
                                           pT[:, kt, cs:512],
                                           start=(kt == 0), stop=(kt == n_kt - 1))
                  if not av_inter:
                      for kt in range(n_kt):
                          j = kt - 4 * qc
                          cs = max(0, j * 128)
                          nc.tensor.matmul(pso[:, cs:512],
                                           v_tiles[kt][:, h, :],
                                           pT[:, kt, cs:512],
                                           start=(kt == 0), stop=(kt == n_kt - 1))
                  recip = smallp.tile([1, 512], F32, tag="recip")
                  nc.vector.reciprocal(recip, pso[64:65, :])
                  recip_b = smallp.tile([64, 512], F32, tag="recipb")
                  nc.gpsimd.partition_broadcast(recip_b, recip, channels=64)
                  if h % 2 == 0:
                      nc.vector.tensor_tensor(oT_tiles[qc][0:64, h // 2, :],
                                              pso[0:64, :], recip_b,
                                              mybir.AluOpType.mult)
                  else:
                      otmp = smallp.tile([64, 512], BF16, tag="otmp")
                      nc.vector.tensor_tensor(otmp, pso[0:64, :], recip_b,
                                              mybir.AluOpType.mult)
                      nc.gpsimd.dma_start(oT_tiles[qc][64:128, h // 2, :], otmp)

              def emit_proj(tt):
                  ts = slice(tt * 128, (tt + 1) * 128)
                  tl = oT_tiles[tt // 4]
                  tsl = slice((tt % 4) * 128, (tt % 4 + 1) * 128)
                  osb = outp.tile([128, DM], F32, tag="osb")
                  for nn in range(2):
                      ns = slice(nn * 512, (nn + 1) * 512)
                      ps2 = psS.tile([128, 2, 512], F32, tag="s")
                      psp = ps2[:, 0, :]
                      nc.tensor.matmul(psp, tl[:, 0, tsl], wp_sb[:, 0, ns],
                                       start=True, stop=False)
                      nc.tensor.matmul(psp, tl[:, 1, tsl], wp_sb[:, 1, ns],
                                       start=False, stop=True)
                      if with_bias:
                          nc.vector.tensor_tensor(osb[:, ns], psp, bias_b[:, ns],
                                                  mybir.AluOpType.add)
                      elif nn == 0:
                          nc.vector.tensor_copy(out=osb[:, ns], in_=psp)
                      else:
                          nc.scalar.copy(out=osb[:, ns], in_=psp)
                  nc.gpsimd.dma_start(out[ts, :], osb)

              # interleaved emission: QKV for the 4 token-tiles of q-chunk qc,
              # then attention for qc, then projection for the tiles of qc-1
              # (proj for qc emitted after attn so its deps are final).
              for qc in range(QC):
                  if phases & 1:
                      for tt in range(4 * qc, 4 * qc + 4):
                          emit_qkv(tt)
                  if phases & 2:
                      for h in range(HPC):
                          emit_attn(qc, h)
                  if phases & 4:
                      for tt in range(4 * qc, 4 * qc + 4):
                          emit_proj(tt)

    nc.finalize()
    return nc


def _rope_tables():
    inv_freq = 1.0 / (MAX_WAVELENGTH ** (np.arange(0, HD, 2, dtype=np.float32) / HD))
    t = np.arange(S, dtype=np.float32)[:, None] * inv_freq[None, :]  # [S, 32]
    emb = np.concatenate([t, t], axis=1)  # [S, 64]
    cos = np.cos(emb).astype(np.float32)
    sin = np.sin(emb).astype(np.float32)
    sin_signed = np.concatenate([-sin[:, :32], sin[:, 32:]], axis=1)
    return cos, sin_signed


def _make_in_maps(x, w_qkv, w_proj, b_proj):
    import ml_dtypes

    x = np.asarray(x, dtype=np.float32)
    w_qkv = np.asarray(w_qkv, dtype=np.float32)
    w_proj = np.asarray(w_proj, dtype=np.float32)
    b_proj = np.asarray(b_proj, dtype=np.float32)

    cos, sin_signed = _rope_tables()
    bf = ml_dtypes.bfloat16

    in_maps = []
    for c in range(NCORES):
        b = c // 4
        g = c % 4
        heads = range(g * HPC, (g + 1) * HPC)
        xT = np.ascontiguousarray(x[b].T).astype(bf)                     # [DM, S]
        wq = np.concatenate([w_qkv[:, h * HD:(h + 1) * HD] for h in heads], axis=1)
        wk = np.concatenate([w_qkv[:, DM + h * HD:DM + (h + 1) * HD] for h in heads], axis=1)
        wvv = np.concatenate([w_qkv[:, 2 * DM + h * HD:2 * DM + (h + 1) * HD] for h in heads], axis=1)
        wqk = np.concatenate([wq, wk], axis=1).astype(bf)                # [DM, 512]
        wvv = wvv.astype(bf)                                             # [DM, 256]
        wpl = w_proj[g * 256:(g + 1) * 256, :].astype(bf)                # [256, DM]
        in_maps.append({
            "xT": xT,
            "wqk": np.ascontiguousarray(wqk),
            "wv": np.ascontiguousarray(wvv),
            "wp": np.ascontiguousarray(wpl),
            "bias4": (b_proj / 4.0).astype(np.float32)[None, :],
            "cos_t": cos,
            "sin_t": sin_signed,
        })
    return in_maps


def kernel(x, w_qkv, w_proj, b_proj):
    from concourse.bass_utils import run_bass_kernel_spmd

    with_bias = bool(np.any(np.asarray(b_proj)))
    key = ("nc", with_bias)
    if key not in _cache:
        _cache[key] = _build_nc(with_bias=with_bias)
    nc = _cache[key]

    in_maps = _make_in_maps(x, w_qkv, w_proj, b_proj)
    res = run_bass_kernel_spmd(nc, in_maps, core_ids=list(range(NCORES)))
    outs = [r["out_partial"] for r in res.results]
    full = np.empty((B, S, DM), dtype=np.float32)
    for b in range(B):
        full[b] = outs[4 * b] + outs[4 * b + 1] + outs[4 * b + 2] + outs[4 * b + 3]
    return full



# revision 6
# speedup vs baseline: 1.3517x; 1.3517x over previous
"""Causal self-attention (B=2, S=2048, dim=1024, 16 heads, RoPE) on 8 trn2 cores.

Sharding: batch x head-group. Core c handles batch c//4 and heads [4*(c%4), 4*(c%4)+4).
QKV is column-parallel, attention embarrassingly parallel per (batch, head), output
projection row-parallel (each core emits a bf16 partial [S, dim] over its heads' 256
attn dims); the host sums the 4 partials per batch and adds b_proj.

Device pipeline per core (matmuls bf16, f32 PSUM accumulation):
  A) QKV: lhsT = x^T tile (host-pretransposed bf16), rhs = w_qkv column slice.
  B) RoPE on Q,K in token-major layout (3 DVE ops using a negative-stride
     half-swap AP and bf16 tables), then ONE XBAR DMA block-transpose per token
     tile moves Q^T/K^T [2h*64, 128] into qkT_all -- no PE transposes, no copies.
  C) Per (head, q-chunk of 512): scores^T = K^T.T @ Q^T chunk -> PSUM pairs,
     exp via ScalarE (scale=1/8 folded; logits O(6) so no max subtraction),
     causal via skipping masked tiles + gpsimd affine_select on diagonal blocks.
     AV reoriented: out[q(128), qs, 65] = P^T-chunk.T @ (V||ones) so the softmax
     denominator lands per-PARTITION: DVE reciprocal + broadcast-mult normalize
     (no gpsimd partition_broadcast). O^T for proj via one XBAR DMA transpose
     per (q-chunk, head-pair).
  D) proj: lhsT = O^T [128, t], rhs = w_proj row-slice; PSUM -> SBUF bf16 via
     DVE; bf16 partial DMA'd out. Startup DMAs ordered wqk -> xT token-chunks
     so the first QKV matmul fires ~6us in.
"""

import sys

sys.path.insert(0, "/opt/trn_rl_repo")

import numpy as np

B = 2
S = 2048
DM = 1024
NH = 16
HD = 64
NCORES = 8
HPC = 4          # heads per core
TT = S // 128    # 16 token tiles
QC = 4           # q-chunks of 512
MAX_WAVELENGTH = 10000.0

_cache = {}


def _build_nc(reps=1):
    import concourse.bass as bass
    import concourse.tile as tile
    import concourse.mybir as mybir
    from concourse import bacc
    from concourse.masks import make_identity

    F32 = mybir.dt.float32
    BF16 = mybir.dt.bfloat16
    Exp = mybir.ActivationFunctionType.Exp

    nc = bacc.Bacc()

    xT = nc.dram_tensor("xT", [DM, S], BF16, kind="ExternalInput")
    wqk = nc.dram_tensor("wqk", [DM, 512], BF16, kind="ExternalInput")
    wv = nc.dram_tensor("wv", [DM, 256], BF16, kind="ExternalInput")
    wp = nc.dram_tensor("wp", [256, DM], BF16, kind="ExternalInput")
    cos_t = nc.dram_tensor("cos_t", [S, HD], BF16, kind="ExternalInput")
    sin_t = nc.dram_tensor("sin_t", [S, HD], BF16, kind="ExternalInput")
    out = nc.dram_tensor("out_partial", [S, DM], BF16, kind="ExternalOutput")

    with tile.TileContext(nc) as tc:
        with tc.tile_pool(name="persist", bufs=1) as persist, \
             tc.tile_pool(name="ropep", bufs=2) as ropep, \
             tc.tile_pool(name="pTp", bufs=2) as pTp, \
             tc.tile_pool(name="onp", bufs=2) as onp, \
             tc.tile_pool(name="smallp", bufs=3) as smallp, \
             tc.tile_pool(name="outp", bufs=3) as outp, \
             tc.tile_pool(name="psB", bufs=2, space="PSUM") as psB, \
             tc.tile_pool(name="psS", bufs=2, space="PSUM") as psS, \
             tc.tile_pool(name="psO", bufs=2, space="PSUM") as psO:
            ident = persist.tile([128, 128], BF16)
            make_identity(nc, ident)

            for _rep in range(reps):
              # --- constant loads: wqk first, then xT token-chunk 0, so the
              # first QKV matmul can start ~6us in; remaining chunks stream
              # behind it.
              wqk_sb = persist.tile([128, 8, 512], BF16)
              nc.sync.dma_start(wqk_sb, wqk.rearrange("(mc p) c -> p mc c", p=128))
              xT_sb = persist.tile([128, 8, S], BF16)
              xTr = xT.rearrange("(mc p) t -> p mc t", p=128)
              nc.sync.dma_start(xT_sb[:, :, 0:512], xTr[:, :, 0:512])
              cos_sb = persist.tile([128, TT, HD], BF16)
              nc.sync.dma_start(cos_sb, cos_t.rearrange("(tt p) d -> p tt d", p=128))
              sin_sb = persist.tile([128, TT, HD], BF16)
              nc.sync.dma_start(sin_sb, sin_t.rearrange("(tt p) d -> p tt d", p=128))
              wv_sb = persist.tile([128, 8, 256], BF16)
              nc.sync.dma_start(wv_sb, wv.rearrange("(mc p) c -> p mc c", p=128))
              for tck in range(1, 4):
                  ts = slice(tck * 512, (tck + 1) * 512)
                  nc.sync.dma_start(xT_sb[:, :, ts], xTr[:, :, ts])
              wp_sb = persist.tile([128, 2, DM], BF16)
              nc.sync.dma_start(wp_sb, wp.rearrange("(kc p) n -> p kc n", p=128))

              # PE warm-up: keep TensorE busy during the initial DMAs so the
              # HAM clock gate is at 2.4 GHz when real matmuls arrive.
              warm = psO.tile([128, 128], BF16, tag="o", name="warm")
              for _w in range(48):
                  nc.tensor.transpose(warm, ident, ident)

              # V in token-major with a ones column per head, one tile per
              # token-tile so attention only depends on the tiles it reads
              v_tiles = {}
              for tt in range(TT):
                  v_tiles[tt] = persist.tile([128, HPC, 65], BF16, tag=f"v_{tt}", name=f"v_{tt}")
                  nc.gpsimd.memset(v_tiles[tt][:, :, 64:65], 1.0)
              # roped Q^T/K^T, written by XBAR DMA transpose.
              # cc: 0=Qh01 1=Qh23 2=Kh01 3=Kh23; [p=64*2h, cc, tokens]
              qkT_all = persist.tile([128, 4, S], BF16, tag="qkT", name="qkT")
              # packed O^T for proj lhsT, per q-chunk: [p=2-head dims, pair, 512]
              oT_tiles = {}
              for qi in range(QC):
                  oT_tiles[qi] = persist.tile([128, 2, 512], BF16, tag=f"oT_{qi}", name=f"oT_{qi}")

              def emit_qkv(tt):
                  ts = slice(tt * 128, (tt + 1) * 128)
                  psqk = psB.tile([128, 512], F32, tag="qk")
                  for mm in range(8):
                      nc.tensor.matmul(psqk, xT_sb[:, mm, ts], wqk_sb[:, mm, :],
                                       start=(mm == 0), stop=(mm == 7))
                  psv_full = psB.tile([128, 512], F32, tag="qk", name="psv")
                  psv = psv_full[:, 0:256]
                  for mm in range(8):
                      nc.tensor.matmul(psv, xT_sb[:, mm, ts], wv_sb[:, mm, :],
                                       start=(mm == 0), stop=(mm == 7))

                  # RoPE over the 8 (4Q + 4K) 64-wide head blocks of psqk:
                  # t_sin = halfswap(psqk) * sin_signed; t_cos = psqk * cos;
                  # qkro = t_cos + t_sin (all-bf16 SBUF add -> DVE 2x mode)
                  pv4 = psqk.rearrange("p (b h s) -> p b h s", b=8, s=32)
                  swapped = pv4[:, :, ::-1, :]
                  t_sin = ropep.tile([128, 512], BF16, tag="tsin")
                  sv = sin_sb[:, tt, :].rearrange("p (h s) -> p h s", s=32)
                  nc.vector.tensor_tensor(
                      t_sin.rearrange("p (b h s) -> p b h s", b=8, s=32),
                      swapped,
                      sv[:, None, :, :].to_broadcast([128, 8, 2, 32]),
                      mybir.AluOpType.mult)
                  t_cos = ropep.tile([128, 512], BF16, tag="tcos")
                  nc.vector.tensor_tensor(
                      t_cos.rearrange("p (b d) -> p b d", b=8),
                      psqk.rearrange("p (b d) -> p b d", b=8),
                      cos_sb[:, tt, None, :].to_broadcast([128, 8, HD]),
                      mybir.AluOpType.mult)
                  qkro = ropep.tile([128, 512], BF16, tag="qkro")
                  nc.vector.tensor_tensor(qkro, t_cos, t_sin, mybir.AluOpType.add)

                  # V copyback (cast to bf16) into the ones-augmented layout
                  nc.vector.tensor_copy(out=v_tiles[tt][:, :, 0:64],
                                        in_=psv.rearrange("p (h d) -> p h d", h=HPC))

                  # one XBAR block transpose: qkT_all[p, cc, t] = qkro[t, cc*128+p]
                  nc.sync.dma_start_transpose(qkT_all[:, :, ts], qkro)

              def av_thunks(qc, h, onorm):
                  """AV + normalize for (qc, h) as a list of emission thunks,
                  to be interleaved between the next head's score pairs so PE
                  has work while ScalarE chews through that head's exps."""
                  pT = pT_tiles[h % 2]
                  pso = psO.tile([128, HPC, 65], F32, tag="o",
                                 name=f"pso_{qc}_{h}")
                  pbase = (h % 2) * 64
                  thunks = []
                  for qs in range(4):
                      n_kt_qs = 4 * qc + qs + 1
                      for kt in range(n_kt_qs):
                          def mm(qs=qs, kt=kt, n=n_kt_qs):
                              nc.tensor.matmul(
                                  pso[:, qs, :],
                                  pT[:, kt, qs * 128:(qs + 1) * 128],
                                  v_tiles[kt][:, h, :],
                                  start=(kt == 0), stop=(kt == n - 1))
                          thunks.append(mm)

                  def norm():
                      recip = smallp.tile([128, 4], F32, tag="recip",
                                          name=f"recip_{qc}_{h}")
                      nc.vector.reciprocal(recip, pso[:, :, 64])
                      nc.vector.tensor_tensor(
                          onorm[:, :, pbase:pbase + 64],
                          pso[:, :, 0:64],
                          recip[:, :, None].to_broadcast([128, 4, 64]),
                          mybir.AluOpType.mult)
                      if h % 2 == 1:  # head pair complete -> O^T via XBAR
                          nc.sync.dma_start_transpose(
                              oT_tiles[qc][:, h // 2, :].rearrange(
                                  "p (a b) -> p a b", a=4),
                              onorm)
                  thunks.append(norm)
                  return thunks

              def emit_attn(qc, h, fillers):
                  """Score pairs + exps for (qc, h), with filler thunks (AV of
                  the previous head, proj tiles) interleaved between pairs."""
                  n_kt = 4 * (qc + 1)
                  n_pairs = n_kt // 2
                  pbase = (h % 2) * 64
                  qT = qkT_all[pbase:pbase + 64, h // 2, qc * 512:(qc + 1) * 512]
                  pT = pTp.tile([128, TT, 512], BF16, tag="pT",
                                name=f"pT_{qc}_{h}")
                  pT_tiles[h % 2] = pT
                  per = -(-len(fillers) // n_pairs) if fillers else 0
                  fi = 0
                  for kp in range(n_pairs):
                      ps2 = psS.tile([128, 2, 512], F32, tag="s",
                                     name=f"s_{qc}_{h}_{kp}")
                      for ki in range(2):
                          kt = kp * 2 + ki
                          j = kt - 4 * qc  # >=0 on diagonal-crossing tiles
                          cs = max(0, j * 128)
                          kT = qkT_all[pbase:pbase + 64, 2 + h // 2,
                                       kt * 128:(kt + 1) * 128]
                          nc.tensor.matmul(
                              ps2[:, ki, cs:512],
                              kT,
                              qT[:, cs:512],
                              start=True, stop=True)
                      if kp * 2 < 4 * qc:  # both tiles full: single big exp
                          nc.scalar.activation(
                              out=pT[:, kp * 2:kp * 2 + 2, :],
                              in_=ps2,
                              func=Exp, scale=0.125)
                      else:
                          for ki in range(2):
                              kt = kp * 2 + ki
                              j = kt - 4 * qc
                              cs = j * 128
                              nc.scalar.activation(out=pT[:, kt, cs:512],
                                                   in_=ps2[:, ki, cs:512],
                                                   func=Exp, scale=0.125)
                      for ki in range(2):
                          kt = kp * 2 + ki
                          j = kt - 4 * qc
                          if j >= 0:
                              blk = slice(j * 128, (j + 1) * 128)
                              nc.gpsimd.affine_select(
                                  out=pT[:, kt, blk], in_=pT[:, kt, blk],
                                  pattern=[[1, 128]], channel_multiplier=-1,
                                  base=0, compare_op=mybir.AluOpType.is_ge,
                                  fill=0.0)
                      for _ in range(per):
                          if fi < len(fillers):
                              fillers[fi]()
                              fi += 1
                  while fi < len(fillers):
                      fillers[fi]()
                      fi += 1

              def emit_proj(tt):
                  # proj PSUM comes from the psB pool (shared with QKV's
                  # psqk/psv rotation) so psS stays dedicated to score pairs.
                  ts = slice(tt * 128, (tt + 1) * 128)
                  tl = oT_tiles[tt // 4]
                  tsl = slice((tt % 4) * 128, (tt % 4 + 1) * 128)
                  osb = outp.tile([128, DM], BF16, tag="osb",
                                  name=f"osb_{tt}")
                  for nn in range(2):
                      ns = slice(nn * 512, (nn + 1) * 512)
                      pj = psB.tile([128, 512], F32, tag="qk",
                                    name=f"pj_{tt}_{nn}")
                      nc.tensor.matmul(pj, tl[:, 0, tsl], wp_sb[:, 0, ns],
                                       start=True, stop=False)
                      nc.tensor.matmul(pj, tl[:, 1, tsl], wp_sb[:, 1, ns],
                                       start=False, stop=True)
                      nc.vector.tensor_copy(out=osb[:, ns], in_=pj)
                  nc.sync.dma_start(out[ts, :], osb)

              # software-pipelined emission: QKV for q-chunk 0 up front; then
              # per (qc, h): one QKV tile of qc+1 ahead of the head's scores,
              # with the previous head's AV+normalize and scheduled proj tiles
              # interleaved between score pairs (PE filler while ScalarE exps).
              proj_sched = {(2, 0): [0], (2, 1): [1], (2, 2): [2], (2, 3): [3],
                            (3, 0): [4, 5], (3, 1): [6, 7],
                            (3, 2): [8, 9], (3, 3): [10, 11]}
              for tt in range(4):
                  emit_qkv(tt)
              pT_tiles = {}
              av_prev = []
              for qc in range(QC):
                  onorms = [onp.tile([128, 4, 128], BF16, tag="on",
                                     name=f"on_{qc}_{i}")
                            for i in range(2)]
                  for h in range(HPC):
                      if qc < QC - 1:
                          emit_qkv(4 * (qc + 1) + h)
                      fillers = list(av_prev)
                      for tt in proj_sched.get((qc, h), []):
                          fillers.append(lambda tt=tt: emit_proj(tt))
                      emit_attn(qc, h, fillers)
                      av_prev = av_thunks(qc, h, onorms[h // 2])
              for th in av_prev:
                  th()
              for tt in range(12, 16):
                  emit_proj(tt)

    nc.finalize()
    return nc


def _rope_tables():
    import ml_dtypes
    inv_freq = 1.0 / (MAX_WAVELENGTH ** (np.arange(0, HD, 2, dtype=np.float32) / HD))
    t = np.arange(S, dtype=np.float32)[:, None] * inv_freq[None, :]  # [S, 32]
    emb = np.concatenate([t, t], axis=1)  # [S, 64]
    cos = np.cos(emb).astype(ml_dtypes.bfloat16)
    sin = np.sin(emb).astype(np.float32)
    sin_signed = np.concatenate([-sin[:, :32], sin[:, 32:]], axis=1).astype(ml_dtypes.bfloat16)
    return cos, sin_signed


def _make_in_maps(x, w_qkv, w_proj):
    import ml_dtypes

    x = np.asarray(x, dtype=np.float32)
    w_qkv = np.asarray(w_qkv, dtype=np.float32)
    w_proj = np.asarray(w_proj, dtype=np.float32)

    cos, sin_signed = _rope_tables()
    bf = ml_dtypes.bfloat16

    in_maps = []
    for c in range(NCORES):
        b = c // 4
        g = c % 4
        heads = range(g * HPC, (g + 1) * HPC)
        xTc = np.ascontiguousarray(x[b].T).astype(bf)                    # [DM, S]
        wq = np.concatenate([w_qkv[:, h * HD:(h + 1) * HD] for h in heads], axis=1)
        wk = np.concatenate([w_qkv[:, DM + h * HD:DM + (h + 1) * HD] for h in heads], axis=1)
        wvv = np.concatenate([w_qkv[:, 2 * DM + h * HD:2 * DM + (h + 1) * HD] for h in heads], axis=1)
        wqkc = np.concatenate([wq, wk], axis=1).astype(bf)               # [DM, 512]
        wvv = wvv.astype(bf)                                             # [DM, 256]
        wpl = w_proj[g * 256:(g + 1) * 256, :].astype(bf)                # [256, DM]
        in_maps.append({
            "xT": xTc,
            "wqk": np.ascontiguousarray(wqkc),
            "wv": np.ascontiguousarray(wvv),
            "wp": np.ascontiguousarray(wpl),
            "cos_t": cos,
            "sin_t": sin_signed,
        })
    return in_maps


def kernel(x, w_qkv, w_proj, b_proj):
    from concourse.bass_utils import run_bass_kernel_spmd

    if "nc" not in _cache:
        _cache["nc"] = _build_nc()
    nc = _cache["nc"]

    in_maps = _make_in_maps(x, w_qkv, w_proj)
    res = run_bass_kernel_spmd(nc, in_maps, core_ids=list(range(NCORES)))
    outs = [r["out_partial"].astype(np.float32) for r in res.results]
    b_proj = np.asarray(b_proj, dtype=np.float32)
    full = np.empty((B, S, DM), dtype=np.float32)
    for b in range(B):
        full[b] = (outs[4 * b] + outs[4 * b + 1] + outs[4 * b + 2]
                   + outs[4 * b + 3]) + b_proj
    return full


# revision 27
# speedup vs baseline: 1.4093x; 1.0426x over previous
"""Causal self-attention (B=2, S=2048, dim=1024, 16 heads, RoPE) on 8 trn2 cores.

Sharding: batch x head-group. Core c handles batch c//4 and heads [4*(c%4), 4*(c%4)+4).
QKV is column-parallel, attention embarrassingly parallel per (batch, head), output
projection row-parallel (each core emits a bf16 partial [S, dim] over its heads' 256
attn dims); the host sums the 4 partials per batch and adds b_proj.

Device pipeline per core (matmuls bf16, f32 PSUM accumulation):
  A) QKV: lhsT = x^T tile (host-pretransposed bf16), rhs = w_qkv column slice.
  B) RoPE on Q,K in token-major layout (3 DVE ops using a negative-stride
     half-swap AP and bf16 tables), then ONE XBAR DMA block-transpose per token
     tile moves Q^T/K^T [2h*64, 128] into qkT_all -- no PE transposes, no copies.
  C) Per (head, q-chunk of 512): scores^T = K^T.T @ Q^T chunk -> PSUM pairs,
     exp via ScalarE (scale=1/8 folded; logits O(6) so no max subtraction),
     causal via skipping masked tiles + gpsimd affine_select on diagonal blocks.
     AV reoriented: out[q(128), qs, 65] = P^T-chunk.T @ (V||ones) so the softmax
     denominator lands per-PARTITION: DVE reciprocal + broadcast-mult normalize
     (no gpsimd partition_broadcast). O^T for proj via one XBAR DMA transpose
     per (q-chunk, head-pair).
  D) proj: lhsT = O^T [128, t], rhs = w_proj row-slice; PSUM -> SBUF bf16 via
     DVE; bf16 partial DMA'd out. Startup DMAs ordered wqk -> xT token-chunks
     so the first QKV matmul fires ~6us in.
"""

import sys

sys.path.insert(0, "/opt/trn_rl_repo")

import numpy as np

B = 2
S = 2048
DM = 1024
NH = 16
HD = 64
NCORES = 8
HPC = 4          # heads per core
TT = S // 128    # 16 token tiles
QC = 4           # q-chunks of 512
MAX_WAVELENGTH = 10000.0

_cache = {}


def _build_nc(reps=1):
    import concourse.bass as bass
    import concourse.tile as tile
    import concourse.mybir as mybir
    from concourse import bacc
    from concourse.masks import make_identity

    F32 = mybir.dt.float32
    BF16 = mybir.dt.bfloat16
    Exp = mybir.ActivationFunctionType.Exp

    nc = bacc.Bacc()

    xT = nc.dram_tensor("xT", [DM, S], BF16, kind="ExternalInput")
    wqk = nc.dram_tensor("wqk", [DM, 512], BF16, kind="ExternalInput")
    wv = nc.dram_tensor("wv", [DM, 256], BF16, kind="ExternalInput")
    wp = nc.dram_tensor("wp", [256, DM], BF16, kind="ExternalInput")
    cos_t = nc.dram_tensor("cos_t", [S, HD], BF16, kind="ExternalInput")
    sin_t = nc.dram_tensor("sin_t", [S, HD], BF16, kind="ExternalInput")
    out = nc.dram_tensor("out_partial", [S, DM], BF16, kind="ExternalOutput")

    with tile.TileContext(nc) as tc:
        with tc.tile_pool(name="persist", bufs=1) as persist, \
             tc.tile_pool(name="ropep", bufs=2) as ropep, \
             tc.tile_pool(name="pTp", bufs=2) as pTp, \
             tc.tile_pool(name="onp", bufs=2) as onp, \
             tc.tile_pool(name="smallp", bufs=3) as smallp, \
             tc.tile_pool(name="outp", bufs=6) as outp, \
             tc.tile_pool(name="psQK", bufs=2, space="PSUM") as psQK, \
             tc.tile_pool(name="psV", bufs=1, space="PSUM") as psV, \
             tc.tile_pool(name="psS", bufs=2, space="PSUM") as psS, \
             tc.tile_pool(name="psO", bufs=1, space="PSUM") as psO:
            ident = persist.tile([128, 128], BF16)
            make_identity(nc, ident)

            for _rep in range(reps):
              # --- constant loads, split so the first QKV matmuls can stream
              # as soon as the first wqk/xT half-chunks land (~2us in).
              wqk_sb = persist.tile([128, 8, 512], BF16)
              wqkr = wqk.rearrange("(mc p) c -> p mc c", p=128)
              xT_sb = persist.tile([128, 8, S], BF16)
              xTr = xT.rearrange("(mc p) t -> p mc t", p=128)
              nc.sync.dma_start(wqk_sb[:, 0:4, :], wqkr[:, 0:4, :])
              nc.sync.dma_start(xT_sb[:, 0:4, 0:512], xTr[:, 0:4, 0:512])
              nc.sync.dma_start(wqk_sb[:, 4:8, :], wqkr[:, 4:8, :])
              nc.sync.dma_start(xT_sb[:, 4:8, 0:512], xTr[:, 4:8, 0:512])
              wv_sb = persist.tile([128, 8, 256], BF16)
              nc.sync.dma_start(wv_sb, wv.rearrange("(mc p) c -> p mc c", p=128))
              cos_sb = persist.tile([128, TT, HD], BF16)
              nc.sync.dma_start(cos_sb, cos_t.rearrange("(tt p) d -> p tt d", p=128))
              sin_sb = persist.tile([128, TT, HD], BF16)
              nc.sync.dma_start(sin_sb, sin_t.rearrange("(tt p) d -> p tt d", p=128))
              for tck in range(1, 4):
                  ts = slice(tck * 512, (tck + 1) * 512)
                  nc.sync.dma_start(xT_sb[:, :, ts], xTr[:, :, ts])
              wp_sb = persist.tile([128, 2, DM], BF16)
              nc.sync.dma_start(wp_sb, wp.rearrange("(kc p) n -> p kc n", p=128))

              # PE warm-up: keep TensorE busy during the initial DMAs so the
              # HAM clock gate is at 2.4 GHz when real matmuls arrive.
              warm = psO.tile([128, 128], BF16, tag="o", name="warm")
              for _w in range(20):
                  nc.tensor.transpose(warm, ident, ident)

              # V in token-major with a ones column per head, one tile per
              # token-tile so attention only depends on the tiles it reads
              v_tiles = {}
              for tt in range(TT):
                  v_tiles[tt] = persist.tile([128, HPC, 65], BF16, tag=f"v_{tt}", name=f"v_{tt}")
                  nc.gpsimd.memset(v_tiles[tt][:, :, 64:65], 1.0)
              # roped Q^T/K^T, written by XBAR DMA transpose.
              # cc: 0=Qh01 1=Qh23 2=Kh01 3=Kh23; [p=64*2h, cc, tokens]
              qkT_all = persist.tile([128, 4, S], BF16, tag="qkT", name="qkT")
              # packed O^T for proj lhsT, per q-chunk: [p=2-head dims, pair, 512]
              oT_tiles = {}
              for qi in range(QC):
                  oT_tiles[qi] = persist.tile([128, 2, 512], BF16, tag=f"oT_{qi}", name=f"oT_{qi}")

              def qkv_mms(tt, psqk, mms):
                  ts = slice(tt * 128, (tt + 1) * 128)
                  for mm in mms:
                      nc.tensor.matmul(psqk, xT_sb[:, mm, ts], wqk_sb[:, mm, :],
                                       start=(mm == 0), stop=(mm == 7))

              def v_mms(tt, psv):
                  ts = slice(tt * 128, (tt + 1) * 128)
                  for mm in range(8):
                      nc.tensor.matmul(psv, xT_sb[:, mm, ts], wv_sb[:, mm, :],
                                       start=(mm == 0), stop=(mm == 7))

              def v_copy(tt, psv):
                  # V copyback (cast to bf16); DVE, after the rope reads
                  nc.vector.tensor_copy(out=v_tiles[tt][:, :, 0:64],
                                        in_=psv.rearrange("p (h d) -> p h d", h=HPC))

              def rope_and_transpose(tt, psqk):
                  # RoPE over the 8 (4Q + 4K) 64-wide head blocks of psqk:
                  # t_sin = halfswap(psqk) * sin_signed; t_cos = psqk * cos;
                  # qkro = t_cos + t_sin (all-bf16 SBUF add -> DVE 2x mode)
                  pv4 = psqk.rearrange("p (b h s) -> p b h s", b=8, s=32)
                  swapped = pv4[:, :, ::-1, :]
                  t_sin = ropep.tile([128, 512], BF16, tag="tsin")
                  sv = sin_sb[:, tt, :].rearrange("p (h s) -> p h s", s=32)
                  nc.vector.tensor_tensor(
                      t_sin.rearrange("p (b h s) -> p b h s", b=8, s=32),
                      swapped,
                      sv[:, None, :, :].to_broadcast([128, 8, 2, 32]),
                      mybir.AluOpType.mult)
                  t_cos = ropep.tile([128, 512], BF16, tag="tcos")
                  nc.vector.tensor_tensor(
                      t_cos.rearrange("p (b d) -> p b d", b=8),
                      psqk.rearrange("p (b d) -> p b d", b=8),
                      cos_sb[:, tt, None, :].to_broadcast([128, 8, HD]),
                      mybir.AluOpType.mult)
                  qkro = ropep.tile([128, 512], BF16, tag="qkro")
                  nc.vector.tensor_tensor(qkro, t_cos, t_sin, mybir.AluOpType.add)

                  # one XBAR block transpose: qkT_all[p, cc, t] = qkro[t, cc*128+p]
                  ts = slice(tt * 128, (tt + 1) * 128)
                  nc.sync.dma_start_transpose(qkT_all[:, :, ts], qkro)

              def emit_qkv(tt):
                  psqk = psQK.tile([128, 512], F32, tag="qk",
                                   name=f"psqk_{tt}")
                  qkv_mms(tt, psqk, range(8))
                  psv = psV.tile([128, 256], F32, tag="v", name=f"psv_{tt}")
                  v_mms(tt, psv)
                  rope_and_transpose(tt, psqk)
                  v_copy(tt, psv)

              def av_thunks(qc, h, onorm):
                  """AV + normalize for (qc, h) as a list of emission thunks,
                  to be interleaved between the next head's score pairs so PE
                  has work while ScalarE chews through that head's exps."""
                  pT = pT_tiles[h % 2]
                  pso = psO.tile([128, HPC, 65], F32, tag="o",
                                 name=f"pso_{qc}_{h}")
                  pbase = (h % 2) * 64
                  thunks = []
                  for qs in range(4):
                      n_kt_qs = 4 * qc + qs + 1
                      for kt in range(n_kt_qs):
                          def mm(qs=qs, kt=kt, n=n_kt_qs):
                              nc.tensor.matmul(
                                  pso[:, qs, :],
                                  pT[:, kt, qs * 128:(qs + 1) * 128],
                                  v_tiles[kt][:, h, :],
                                  start=(kt == 0), stop=(kt == n - 1))
                          thunks.append((27, mm))

                  if qc == QC - 1 and h == HPC - 1:
                      # final head: normalize + transpose per q-subchunk so
                      # each tail proj tile starts as soon as its slice lands
                      def norm_qs(qs):
                          recip = smallp.tile([128, 1], F32, tag="recip",
                                              name=f"recip_{qc}_{h}_{qs}")
                          nc.vector.reciprocal(recip, pso[:, qs, 64:65])
                          nc.vector.tensor_tensor(
                              onorm[:, qs, pbase:pbase + 64],
                              pso[:, qs, 0:64],
                              recip[:, :].to_broadcast([128, 64]),
                              mybir.AluOpType.mult)
                          nc.sync.dma_start_transpose(
                              oT_tiles[qc][:, h // 2, qs * 128:(qs + 1) * 128],
                              onorm[:, qs, :])
                      # insert each norm right after its qs chain's last matmul
                      out_thunks = []
                      i = 0
                      for qs in range(4):
                          n_kt_qs = 4 * qc + qs + 1
                          out_thunks.extend(thunks[i:i + n_kt_qs])
                          i += n_kt_qs
                          out_thunks.append((50, lambda qs=qs: norm_qs(qs)))
                      return out_thunks

                  def norm():
                      recip = smallp.tile([128, 4], F32, tag="recip",
                                          name=f"recip_{qc}_{h}")
                      nc.vector.reciprocal(recip, pso[:, :, 64])
                      nc.vector.tensor_tensor(
                          onorm[:, :, pbase:pbase + 64],
                          pso[:, :, 0:64],
                          recip[:, :, None].to_broadcast([128, 4, 64]),
                          mybir.AluOpType.mult)
                      if h % 2 == 1:  # head pair complete -> O^T via XBAR
                          nc.sync.dma_start_transpose(
                              oT_tiles[qc][:, h // 2, :].rearrange(
                                  "p (a b) -> p a b", a=4),
                              onorm)
                  thunks.append((50, norm))
                  return thunks

              def emit_attn(qc, h, fillers):
                  """Score pairs + exps for (qc, h), with filler thunks (AV of
                  the previous head, proj tiles) interleaved between pairs."""
                  n_kt = 4 * (qc + 1)
                  n_pairs = n_kt // 2
                  pbase = (h % 2) * 64
                  qT = qkT_all[pbase:pbase + 64, h // 2, qc * 512:(qc + 1) * 512]
                  pT = pTp.tile([128, TT, 512], BF16, tag="pT",
                                name=f"pT_{qc}_{h}")
                  pT_tiles[h % 2] = pT
                  # deal filler thunks between pairs weighted by their PE cost
                  # so each inter-pair slot gets roughly equal fill time
                  total_cost = sum(c for c, _ in fillers)
                  done_cost = 0.0
                  fi = 0
                  for kp in range(n_pairs):
                      ps2 = psS.tile([128, 2, 512], F32, tag="s",
                                     name=f"s_{qc}_{h}_{kp}")
                      for ki in range(2):
                          kt = kp * 2 + ki
                          j = kt - 4 * qc  # >=0 on diagonal-crossing tiles
                          cs = max(0, j * 128)
                          kT = qkT_all[pbase:pbase + 64, 2 + h // 2,
                                       kt * 128:(kt + 1) * 128]
                          nc.tensor.matmul(
                              ps2[:, ki, cs:512],
                              kT,
                              qT[:, cs:512],
                              start=True, stop=True)
                      if kp * 2 < 4 * qc:  # both tiles full: single big exp
                          nc.scalar.activation(
                              out=pT[:, kp * 2:kp * 2 + 2, :],
                              in_=ps2,
                              func=Exp, scale=0.125)
                      else:
                          for ki in range(2):
                              kt = kp * 2 + ki
                              j = kt - 4 * qc
                              cs = j * 128
                              nc.scalar.activation(out=pT[:, kt, cs:512],
                                                   in_=ps2[:, ki, cs:512],
                                                   func=Exp, scale=0.125)
                      for ki in range(2):
                          kt = kp * 2 + ki
                          j = kt - 4 * qc
                          if j >= 0:
                              blk = slice(j * 128, (j + 1) * 128)
                              nc.gpsimd.affine_select(
                                  out=pT[:, kt, blk], in_=pT[:, kt, blk],
                                  pattern=[[1, 128]], channel_multiplier=-1,
                                  base=0, compare_op=mybir.AluOpType.is_ge,
                                  fill=0.0)
                      quota = total_cost * (kp + 1) / n_pairs
                      while fi < len(fillers) and done_cost < quota:
                          done_cost += fillers[fi][0]
                          fillers[fi][1]()
                          fi += 1
                  while fi < len(fillers):
                      fillers[fi][1]()
                      fi += 1

              def proj_half(tt, nn, osb, tail):
                  # proj PSUM comes from the psQK pool (shared with the QKV
                  # psqk rotation) so psS stays dedicated to score pairs.
                  # Tail tiles split copies across Act (idle then) and DVE.
                  ts = slice(tt * 128, (tt + 1) * 128)
                  tl = oT_tiles[tt // 4]
                  tsl = slice((tt % 4) * 128, (tt % 4 + 1) * 128)
                  ns = slice(nn * 512, (nn + 1) * 512)
                  pj = psQK.tile([128, 512], F32, tag="qk",
                                 name=f"pj_{tt}_{nn}")
                  nc.tensor.matmul(pj, tl[:, 0, tsl], wp_sb[:, 0, ns],
                                   start=True, stop=False)
                  nc.tensor.matmul(pj, tl[:, 1, tsl], wp_sb[:, 1, ns],
                                   start=False, stop=True)
                  if tail and nn == 0:
                      nc.scalar.copy(out=osb[:, ns], in_=pj)
                  else:
                      nc.vector.tensor_copy(out=osb[:, ns], in_=pj)
                  if tail:  # fire each half as soon as its copy lands
                      nc.sync.dma_start(out[ts, ns], osb[:, ns])
                  elif nn == 1:
                      nc.sync.dma_start(out[ts, :], osb)

              def proj_thunks(tt, tail=False):
                  osb = outp.tile([128, DM], BF16, tag="osb",
                                  name=f"osb_{tt}")
                  return [(430, lambda nn=nn: proj_half(tt, nn, osb, tail))
                          for nn in range(2)]

              def emit_proj(tt, tail=False):
                  for _, th in proj_thunks(tt, tail):
                      th()

              # software-pipelined emission: QKV for q-chunk 0 up front; then
              # per (qc, h): one QKV tile of qc+1 ahead of the head's scores,
              # with the previous head's AV+normalize and scheduled proj tiles
              # interleaved between score pairs (PE filler while ScalarE exps).
              # all deferrable proj work lands in qc3's cycles, where ScalarE's
              # exp hump would otherwise leave PE idle; QKV fillers finish by
              # h2 so the next chunk's last rope+transpose beats scores(qc+1,h0)
              proj_sched = {(3, 0): [0, 1, 2], (3, 1): [3, 4, 5],
                            (3, 2): [6, 7, 8], (3, 3): [9, 10, 11]}
              qkv_sched = {0: [0], 1: [1], 2: [2, 3]}
              # startup: stream tiles 0/1 mm-major in wqk/xT half-chunk order
              # so PE follows the arriving DMA halves instead of waiting for
              # the full 2MB; tiles 2/3 go through the normal path.
              psqk_s = {tt: psQK.tile([128, 512], F32, tag="qk",
                                      name=f"psqk_s{tt}")
                        for tt in (0, 1)}
              for tt in (0, 1):
                  qkv_mms(tt, psqk_s[tt], range(0, 4))
              for tt in (0, 1):
                  qkv_mms(tt, psqk_s[tt], range(4, 8))
              rope_and_transpose(0, psqk_s[0])
              psv_s0 = psV.tile([128, 256], F32, tag="v", name="psv_s0")
              v_mms(0, psv_s0)
              rope_and_transpose(1, psqk_s[1])
              v_copy(0, psv_s0)
              psv_s1 = psV.tile([128, 256], F32, tag="v", name="psv_s1")
              v_mms(1, psv_s1)
              v_copy(1, psv_s1)
              emit_qkv(2)
              emit_qkv(3)
              pT_tiles = {}
              av_prev = []
              for qc in range(QC):
                  onorms = [onp.tile([128, 4, 128], BF16, tag="on",
                                     name=f"on_{qc}_{i}")
                            for i in range(2)]
                  for h in range(HPC):
                      if qc < QC - 1:
                          for dt in qkv_sched.get(h, []):
                              emit_qkv(4 * (qc + 1) + dt)
                      fillers = list(av_prev)
                      for tt in proj_sched.get((qc, h), []):
                          fillers.extend(proj_thunks(tt))
                      emit_attn(qc, h, fillers)
                      av_prev = av_thunks(qc, h, onorms[h // 2])
              for th in av_prev:
                  th()
              for tt in range(12, 16):
                  emit_proj(tt, tail=True)

    nc.finalize()
    return nc


def _rope_tables():
    import ml_dtypes
    inv_freq = 1.0 / (MAX_WAVELENGTH ** (np.arange(0, HD, 2, dtype=np.float32) / HD))
    t = np.arange(S, dtype=np.float32)[:, None] * inv_freq[None, :]  # [S, 32]
    emb = np.concatenate([t, t], axis=1)  # [S, 64]
    cos = np.cos(emb).astype(ml_dtypes.bfloat16)
    sin = np.sin(emb).astype(np.float32)
    sin_signed = np.concatenate([-sin[:, :32], sin[:, 32:]], axis=1).astype(ml_dtypes.bfloat16)
    return cos, sin_signed


def _make_in_maps(x, w_qkv, w_proj):
    import ml_dtypes

    x = np.asarray(x, dtype=np.float32)
    w_qkv = np.asarray(w_qkv, dtype=np.float32)
    w_proj = np.asarray(w_proj, dtype=np.float32)

    cos, sin_signed = _rope_tables()
    bf = ml_dtypes.bfloat16

    in_maps = []
    for c in range(NCORES):
        b = c // 4
        g = c % 4
        heads = range(g * HPC, (g + 1) * HPC)
        xTc = np.ascontiguousarray(x[b].T).astype(bf)                    # [DM, S]
        wq = np.concatenate([w_qkv[:, h * HD:(h + 1) * HD] for h in heads], axis=1)
        wk = np.concatenate([w_qkv[:, DM + h * HD:DM + (h + 1) * HD] for h in heads], axis=1)
        wvv = np.concatenate([w_qkv[:, 2 * DM + h * HD:2 * DM + (h + 1) * HD] for h in heads], axis=1)
        wqkc = np.concatenate([wq, wk], axis=1).astype(bf)               # [DM, 512]
        wvv = wvv.astype(bf)                                             # [DM, 256]
        wpl = w_proj[g * 256:(g + 1) * 256, :].astype(bf)                # [256, DM]
        in_maps.append({
            "xT": xTc,
            "wqk": np.ascontiguousarray(wqkc),
            "wv": np.ascontiguousarray(wvv),
            "wp": np.ascontiguousarray(wpl),
            "cos_t": cos,
            "sin_t": sin_signed,
        })
    return in_maps


def kernel(x, w_qkv, w_proj, b_proj):
    from concourse.bass_utils import run_bass_kernel_spmd

    if "nc" not in _cache:
        _cache["nc"] = _build_nc()
    nc = _cache["nc"]

    in_maps = _make_in_maps(x, w_qkv, w_proj)
    res = run_bass_kernel_spmd(nc, in_maps, core_ids=list(range(NCORES)))
    outs = [r["out_partial"].astype(np.float32) for r in res.results]
    b_proj = np.asarray(b_proj, dtype=np.float32)
    full = np.empty((B, S, DM), dtype=np.float32)
    for b in range(B):
        full[b] = (outs[4 * b] + outs[4 * b + 1] + outs[4 * b + 2]
                   + outs[4 * b + 3]) + b_proj
    return full


# revision 34
# speedup vs baseline: 1.4510x; 1.0296x over previous
"""Causal self-attention (B=2, S=2048, dim=1024, 16 heads, RoPE) on 8 trn2 cores.

Sharding: batch x head-group. Core c handles batch c//4 and heads [4*(c%4), 4*(c%4)+4).
QKV is column-parallel, attention embarrassingly parallel per (batch, head), output
projection row-parallel (each core emits a bf16 partial [S, dim] over its heads' 256
attn dims); the host sums the 4 partials per batch and adds b_proj.

Device pipeline per core (matmuls bf16, f32 PSUM accumulation):
  A) QKV: lhsT = x^T tile (host-pretransposed bf16), rhs = w_qkv column slice.
  B) RoPE on Q,K in token-major layout (3 DVE ops using a negative-stride
     half-swap AP and bf16 tables), then ONE XBAR DMA block-transpose per token
     tile moves Q^T/K^T [2h*64, 128] into qkT_all -- no PE transposes, no copies.
  C) Per (head, q-chunk of 512): scores^T = K^T.T @ Q^T chunk -> PSUM pairs,
     exp via ScalarE (scale=1/8 folded; logits O(6) so no max subtraction),
     causal via skipping masked tiles + gpsimd affine_select on diagonal blocks.
     AV reoriented: out[q(128), qs, 65] = P^T-chunk.T @ (V||ones) so the softmax
     denominator lands per-PARTITION: DVE reciprocal + broadcast-mult normalize
     (no gpsimd partition_broadcast). O^T for proj via one XBAR DMA transpose
     per (q-chunk, head-pair).
  D) proj: lhsT = O^T [128, t], rhs = w_proj row-slice; PSUM -> SBUF bf16 via
     DVE; bf16 partial DMA'd out. Startup DMAs ordered wqk -> xT token-chunks
     so the first QKV matmul fires ~6us in.
"""

import sys

sys.path.insert(0, "/opt/trn_rl_repo")

import numpy as np

B = 2
S = 2048
DM = 1024
NH = 16
HD = 64
NCORES = 8
HPC = 4          # heads per core
TT = S // 128    # 16 token tiles
QC = 4           # q-chunks of 512
MAX_WAVELENGTH = 10000.0

_cache = {}


def _build_nc(reps=1):
    import concourse.bass as bass
    import concourse.tile as tile
    import concourse.mybir as mybir
    from concourse import bacc
    from concourse.masks import make_identity

    F32 = mybir.dt.float32
    BF16 = mybir.dt.bfloat16
    Exp = mybir.ActivationFunctionType.Exp

    nc = bacc.Bacc()

    xT = nc.dram_tensor("xT", [DM, S], BF16, kind="ExternalInput")
    wqk = nc.dram_tensor("wqk", [DM, 512], BF16, kind="ExternalInput")
    wv = nc.dram_tensor("wv", [DM, 256], BF16, kind="ExternalInput")
    wp = nc.dram_tensor("wp", [256, DM], BF16, kind="ExternalInput")
    cos_t = nc.dram_tensor("cos_t", [S, HD], BF16, kind="ExternalInput")
    sin_t = nc.dram_tensor("sin_t", [S, HD], BF16, kind="ExternalInput")
    out = nc.dram_tensor("out_partial", [S, DM], BF16, kind="ExternalOutput")

    with tile.TileContext(nc) as tc:
        with tc.tile_pool(name="persist", bufs=1) as persist, \
             tc.tile_pool(name="ropep", bufs=2) as ropep, \
             tc.tile_pool(name="pTp", bufs=2) as pTp, \
             tc.tile_pool(name="onp", bufs=2) as onp, \
             tc.tile_pool(name="smallp", bufs=3) as smallp, \
             tc.tile_pool(name="outp", bufs=6) as outp, \
             tc.tile_pool(name="psQK", bufs=2, space="PSUM") as psQK, \
             tc.tile_pool(name="psV", bufs=1, space="PSUM") as psV, \
             tc.tile_pool(name="psS", bufs=2, space="PSUM") as psS, \
             tc.tile_pool(name="psO", bufs=1, space="PSUM") as psO:
            ident = persist.tile([128, 128], BF16)
            make_identity(nc, ident)

            for _rep in range(reps):
              # --- constant loads, split so the first QKV matmuls can stream
              # as soon as the first wqk/xT half-chunks land (~2us in).
              wqk_sb = persist.tile([128, 8, 512], BF16)
              wqkr = wqk.rearrange("(mc p) c -> p mc c", p=128)
              xT_sb = persist.tile([128, 8, S], BF16)
              xTr = xT.rearrange("(mc p) t -> p mc t", p=128)
              nc.sync.dma_start(wqk_sb[:, 0:4, :], wqkr[:, 0:4, :])
              nc.sync.dma_start(xT_sb[:, 0:4, 0:512], xTr[:, 0:4, 0:512])
              nc.sync.dma_start(wqk_sb[:, 4:8, :], wqkr[:, 4:8, :])
              nc.sync.dma_start(xT_sb[:, 4:8, 0:512], xTr[:, 4:8, 0:512])
              wv_sb = persist.tile([128, 8, 256], BF16)
              nc.sync.dma_start(wv_sb, wv.rearrange("(mc p) c -> p mc c", p=128))
              cos_sb = persist.tile([128, TT, HD], BF16)
              nc.sync.dma_start(cos_sb, cos_t.rearrange("(tt p) d -> p tt d", p=128))
              sin_sb = persist.tile([128, TT, HD], BF16)
              nc.sync.dma_start(sin_sb, sin_t.rearrange("(tt p) d -> p tt d", p=128))
              for tck in range(1, 4):
                  ts = slice(tck * 512, (tck + 1) * 512)
                  nc.sync.dma_start(xT_sb[:, :, ts], xTr[:, :, ts])
              wp_sb = persist.tile([128, 2, DM], BF16)
              nc.sync.dma_start(wp_sb, wp.rearrange("(kc p) n -> p kc n", p=128))

              # PE warm-up: keep TensorE busy during the initial DMAs so the
              # HAM clock gate is at 2.4 GHz when real matmuls arrive.
              warm = psO.tile([128, 128], BF16, tag="o", name="warm")
              for _w in range(20):
                  nc.tensor.transpose(warm, ident, ident)

              # V in token-major with a ones column per head, one tile per
              # token-tile so attention only depends on the tiles it reads
              v_tiles = {}
              for tt in range(TT):
                  v_tiles[tt] = persist.tile([128, HPC, 65], BF16, tag=f"v_{tt}", name=f"v_{tt}")
                  nc.gpsimd.memset(v_tiles[tt][:, :, 64:65], 1.0)
              # roped Q^T/K^T, written by XBAR DMA transpose.
              # cc: 0=Qh01 1=Qh23 2=Kh01 3=Kh23; [p=64*2h, cc, tokens]
              qkT_all = persist.tile([128, 4, S], BF16, tag="qkT", name="qkT")
              # packed O^T for proj lhsT, per q-chunk: [p=2-head dims, pair, 512]
              oT_tiles = {}
              for qi in range(QC):
                  oT_tiles[qi] = persist.tile([128, 2, 512], BF16, tag=f"oT_{qi}", name=f"oT_{qi}")

              def qkv_mms(tt, psqk, mms):
                  ts = slice(tt * 128, (tt + 1) * 128)
                  for mm in mms:
                      nc.tensor.matmul(psqk, xT_sb[:, mm, ts], wqk_sb[:, mm, :],
                                       start=(mm == 0), stop=(mm == 7))

              def v_mms(tt, psv):
                  ts = slice(tt * 128, (tt + 1) * 128)
                  for mm in range(8):
                      nc.tensor.matmul(psv, xT_sb[:, mm, ts], wv_sb[:, mm, :],
                                       start=(mm == 0), stop=(mm == 7))

              def v_copy(tt, psv, on_act=False):
                  # V copyback (cast to bf16); Act for the startup burst
                  # (no exps queued yet, keeps DVE free for the rope chain),
                  # DVE in steady state
                  dst = v_tiles[tt][:, :, 0:64]
                  src = psv.rearrange("p (h d) -> p h d", h=HPC)
                  if on_act:
                      nc.scalar.copy(out=dst, in_=src)
                  else:
                      nc.vector.tensor_copy(out=dst, in_=src)

              def rope_and_transpose(tt, psqk):
                  # RoPE over the 8 (4Q + 4K) 64-wide head blocks of psqk:
                  # t_sin = halfswap(psqk) * sin_signed; t_cos = psqk * cos;
                  # qkro = t_cos + t_sin (all-bf16 SBUF add -> DVE 2x mode)
                  pv4 = psqk.rearrange("p (b h s) -> p b h s", b=8, s=32)
                  swapped = pv4[:, :, ::-1, :]
                  t_sin = ropep.tile([128, 512], BF16, tag="tsin")
                  sv = sin_sb[:, tt, :].rearrange("p (h s) -> p h s", s=32)
                  nc.vector.tensor_tensor(
                      t_sin.rearrange("p (b h s) -> p b h s", b=8, s=32),
                      swapped,
                      sv[:, None, :, :].to_broadcast([128, 8, 2, 32]),
                      mybir.AluOpType.mult)
                  t_cos = ropep.tile([128, 512], BF16, tag="tcos")
                  nc.vector.tensor_tensor(
                      t_cos.rearrange("p (b d) -> p b d", b=8),
                      psqk.rearrange("p (b d) -> p b d", b=8),
                      cos_sb[:, tt, None, :].to_broadcast([128, 8, HD]),
                      mybir.AluOpType.mult)
                  qkro = ropep.tile([128, 512], BF16, tag="qkro")
                  nc.vector.tensor_tensor(qkro, t_cos, t_sin, mybir.AluOpType.add)

                  # one XBAR block transpose: qkT_all[p, cc, t] = qkro[t, cc*128+p]
                  ts = slice(tt * 128, (tt + 1) * 128)
                  nc.sync.dma_start_transpose(qkT_all[:, :, ts], qkro)

              def emit_qkv(tt):
                  psqk = psQK.tile([128, 512], F32, tag="qk",
                                   name=f"psqk_{tt}")
                  qkv_mms(tt, psqk, range(8))
                  psv = psV.tile([128, 256], F32, tag="v", name=f"psv_{tt}")
                  v_mms(tt, psv)
                  rope_and_transpose(tt, psqk)
                  v_copy(tt, psv)

              def qkv_thunks(tt):
                  """QKV for one tile as weighted filler thunks (fine-grained
                  mm units so conservative dealing can place them)."""
                  psqk = psQK.tile([128, 512], F32, tag="qk",
                                   name=f"psqk_f{tt}")
                  psv = psV.tile([128, 256], F32, tag="v", name=f"psv_f{tt}")
                  th = []
                  for mm in range(8):
                      th.append((213, lambda mm=mm: qkv_mms(tt, psqk, [mm])))
                  th.append((50, lambda: rope_and_transpose(tt, psqk)))
                  for mm in range(8):
                      def vmm(mm=mm):
                          ts2 = slice(tt * 128, (tt + 1) * 128)
                          nc.tensor.matmul(psv, xT_sb[:, mm, ts2],
                                           wv_sb[:, mm, :],
                                           start=(mm == 0), stop=(mm == 7))
                      th.append((107, vmm))
                  th.append((50, lambda: v_copy(tt, psv)))
                  return th

              def av_thunks(qc, h, onorm):
                  """AV + normalize for (qc, h) as a list of emission thunks,
                  to be interleaved between the next head's score pairs so PE
                  has work while ScalarE chews through that head's exps."""
                  pT = pT_tiles[h % 2]
                  pso = psO.tile([128, HPC, 65], F32, tag="o",
                                 name=f"pso_{qc}_{h}")
                  pbase = (h % 2) * 64
                  thunks = []
                  for qs in range(4):
                      n_kt_qs = 4 * qc + qs + 1
                      for kt in range(n_kt_qs):
                          def mm(qs=qs, kt=kt, n=n_kt_qs):
                              nc.tensor.matmul(
                                  pso[:, qs, :],
                                  pT[:, kt, qs * 128:(qs + 1) * 128],
                                  v_tiles[kt][:, h, :],
                                  start=(kt == 0), stop=(kt == n - 1))
                          thunks.append((27, mm))

                  if qc == QC - 1 and h == HPC - 1:
                      # final head: normalize + transpose per q-subchunk so
                      # each tail proj tile starts as soon as its slice lands
                      def norm_qs(qs):
                          recip = smallp.tile([128, 1], F32, tag="recip",
                                              name=f"recip_{qc}_{h}_{qs}")
                          nc.vector.reciprocal(recip, pso[:, qs, 64:65])
                          nc.vector.tensor_tensor(
                              onorm[:, qs, pbase:pbase + 64],
                              pso[:, qs, 0:64],
                              recip[:, :].to_broadcast([128, 64]),
                              mybir.AluOpType.mult)
                          nc.sync.dma_start_transpose(
                              oT_tiles[qc][:, h // 2, qs * 128:(qs + 1) * 128],
                              onorm[:, qs, :])
                      # insert each norm right after its qs chain's last matmul
                      out_thunks = []
                      i = 0
                      for qs in range(4):
                          n_kt_qs = 4 * qc + qs + 1
                          out_thunks.extend(thunks[i:i + n_kt_qs])
                          i += n_kt_qs
                          out_thunks.append((50, lambda qs=qs: norm_qs(qs)))
                      return out_thunks

                  def norm():
                      recip = smallp.tile([128, 4], F32, tag="recip",
                                          name=f"recip_{qc}_{h}")
                      nc.vector.reciprocal(recip, pso[:, :, 64])
                      nc.vector.tensor_tensor(
                          onorm[:, :, pbase:pbase + 64],
                          pso[:, :, 0:64],
                          recip[:, :, None].to_broadcast([128, 4, 64]),
                          mybir.AluOpType.mult)
                      if h % 2 == 1:  # head pair complete -> O^T via XBAR
                          nc.sync.dma_start_transpose(
                              oT_tiles[qc][:, h // 2, :].rearrange(
                                  "p (a b) -> p a b", a=4),
                              onorm)
                  thunks.append((50, norm))
                  return thunks

              def emit_attn(qc, h, fillers):
                  """Score pairs + exps for (qc, h), with filler thunks (AV of
                  the previous head, proj tiles) interleaved between pairs."""
                  n_kt = 4 * (qc + 1)
                  n_pairs = n_kt // 2
                  pbase = (h % 2) * 64
                  qT = qkT_all[pbase:pbase + 64, h // 2, qc * 512:(qc + 1) * 512]
                  pT = pTp.tile([128, TT, 512], BF16, tag="pT",
                                name=f"pT_{qc}_{h}")
                  pT_tiles[h % 2] = pT
                  # deal filler thunks between pairs weighted by their PE cost
                  # so each inter-pair slot gets roughly equal fill time
                  total_cost = sum(c for c, _ in fillers)
                  done_cost = 0.0
                  fi = 0
                  for kp in range(n_pairs):
                      ps2 = psS.tile([128, 2, 512], F32, tag="s",
                                     name=f"s_{qc}_{h}_{kp}")
                      for ki in range(2):
                          kt = kp * 2 + ki
                          j = kt - 4 * qc  # >=0 on diagonal-crossing tiles
                          cs = max(0, j * 128)
                          kT = qkT_all[pbase:pbase + 64, 2 + h // 2,
                                       kt * 128:(kt + 1) * 128]
                          nc.tensor.matmul(
                              ps2[:, ki, cs:512],
                              kT,
                              qT[:, cs:512],
                              start=True, stop=True)
                      if kp * 2 < 4 * qc:  # both tiles full: single big exp
                          nc.scalar.activation(
                              out=pT[:, kp * 2:kp * 2 + 2, :],
                              in_=ps2,
                              func=Exp, scale=0.125)
                      else:
                          for ki in range(2):
                              kt = kp * 2 + ki
                              j = kt - 4 * qc
                              cs = j * 128
                              nc.scalar.activation(out=pT[:, kt, cs:512],
                                                   in_=ps2[:, ki, cs:512],
                                                   func=Exp, scale=0.125)
                      for ki in range(2):
                          kt = kp * 2 + ki
                          j = kt - 4 * qc
                          if j >= 0:
                              blk = slice(j * 128, (j + 1) * 128)
                              nc.gpsimd.affine_select(
                                  out=pT[:, kt, blk], in_=pT[:, kt, blk],
                                  pattern=[[1, 128]], channel_multiplier=-1,
                                  base=0, compare_op=mybir.AluOpType.is_ge,
                                  fill=0.0)
                      # conservative fill: never overshoot the slot quota, so
                      # score pairs are not delayed past psS readiness and the
                      # ScalarE exp cadence (the binding rate late in the
                      # kernel) is preserved; leftovers run after the loop
                      quota = total_cost * (kp + 1) / n_pairs
                      while (fi < len(fillers)
                             and done_cost + fillers[fi][0] <= quota):
                          done_cost += fillers[fi][0]
                          fillers[fi][1]()
                          fi += 1
                  while fi < len(fillers):
                      fillers[fi][1]()
                      fi += 1

              def proj_half(tt, nn, osb, tail):
                  # proj PSUM comes from the psQK pool (shared with the QKV
                  # psqk rotation) so psS stays dedicated to score pairs.
                  # Tail tiles split copies across Act (idle then) and DVE.
                  ts = slice(tt * 128, (tt + 1) * 128)
                  tl = oT_tiles[tt // 4]
                  tsl = slice((tt % 4) * 128, (tt % 4 + 1) * 128)
                  ns = slice(nn * 512, (nn + 1) * 512)
                  pj = psQK.tile([128, 512], F32, tag="qk",
                                 name=f"pj_{tt}_{nn}")
                  nc.tensor.matmul(pj, tl[:, 0, tsl], wp_sb[:, 0, ns],
                                   start=True, stop=False)
                  nc.tensor.matmul(pj, tl[:, 1, tsl], wp_sb[:, 1, ns],
                                   start=False, stop=True)
                  if tail and nn == 0:
                      nc.scalar.copy(out=osb[:, ns], in_=pj)
                  else:
                      nc.vector.tensor_copy(out=osb[:, ns], in_=pj)
                  if tail:  # fire each half as soon as its copy lands
                      nc.sync.dma_start(out[ts, ns], osb[:, ns])
                  elif nn == 1:
                      nc.sync.dma_start(out[ts, :], osb)

              def proj_thunks(tt, tail=False):
                  osb = outp.tile([128, DM], BF16, tag="osb",
                                  name=f"osb_{tt}")
                  return [(430, lambda nn=nn: proj_half(tt, nn, osb, tail))
                          for nn in range(2)]

              def emit_proj(tt, tail=False):
                  for _, th in proj_thunks(tt, tail):
                      th()

              # software-pipelined emission: QKV for q-chunk 0 up front; then
              # per (qc, h): one QKV tile of qc+1 ahead of the head's scores,
              # with the previous head's AV+normalize and scheduled proj tiles
              # interleaved between score pairs (PE filler while ScalarE exps).
              # all deferrable proj work lands in qc3's cycles, where ScalarE's
              # exp hump would otherwise leave PE idle; QKV fillers finish by
              # h2 so the next chunk's last rope+transpose beats scores(qc+1,h0)
              proj_sched = {(3, 0): [0, 1, 2], (3, 1): [3, 4, 5],
                            (3, 2): [6, 7, 8], (3, 3): [9, 10, 11]}
              qkv_sched = {0: [0], 1: [1], 2: [2]}
              # startup: stream tiles 0/1 mm-major in wqk/xT half-chunk order
              # so PE follows the arriving DMA halves instead of waiting for
              # the full 2MB; tiles 2/3 go through the normal path.
              psqk_s = {tt: psQK.tile([128, 512], F32, tag="qk",
                                      name=f"psqk_s{tt}")
                        for tt in (0, 1)}
              for tt in (0, 1):
                  qkv_mms(tt, psqk_s[tt], range(0, 4))
              for tt in (0, 1):
                  qkv_mms(tt, psqk_s[tt], range(4, 8))
              rope_and_transpose(0, psqk_s[0])
              psv_s0 = psV.tile([128, 256], F32, tag="v", name="psv_s0")
              v_mms(0, psv_s0)
              rope_and_transpose(1, psqk_s[1])
              v_copy(0, psv_s0, on_act=True)
              psv_s1 = psV.tile([128, 256], F32, tag="v", name="psv_s1")
              v_mms(1, psv_s1)
              v_copy(1, psv_s1, on_act=True)
              for tt in (2, 3):
                  psqk = psQK.tile([128, 512], F32, tag="qk",
                                   name=f"psqk_s{tt}")
                  qkv_mms(tt, psqk, range(8))
                  psv = psV.tile([128, 256], F32, tag="v", name=f"psv_s{tt}")
                  v_mms(tt, psv)
                  rope_and_transpose(tt, psqk)
                  v_copy(tt, psv, on_act=True)
              pT_tiles = {}
              av_prev = []
              for qc in range(QC):
                  onorms = [onp.tile([128, 4, 128], BF16, tag="on",
                                     name=f"on_{qc}_{i}")
                            for i in range(2)]
                  for h in range(HPC):
                      if qc < QC - 1:
                          for dt in qkv_sched.get(h, []):
                              emit_qkv(4 * (qc + 1) + dt)
                      fillers = list(av_prev)
                      if h == 3 and qc < QC - 1:
                          # 4th QKV tile of the next chunk rides as fillers so
                          # its rope+transpose complete before scores(qc+1,h0)
                          fillers.extend(qkv_thunks(4 * (qc + 1) + 3))
                      for tt in proj_sched.get((qc, h), []):
                          fillers.extend(proj_thunks(tt))
                      emit_attn(qc, h, fillers)
                      av_prev = av_thunks(qc, h, onorms[h // 2])
              for _, th in av_prev:
                  th()
              for tt in range(12, 16):
                  emit_proj(tt, tail=True)

    nc.finalize()
    return nc


def _rope_tables():
    import ml_dtypes
    inv_freq = 1.0 / (MAX_WAVELENGTH ** (np.arange(0, HD, 2, dtype=np.float32) / HD))
    t = np.arange(S, dtype=np.float32)[:, None] * inv_freq[None, :]  # [S, 32]
    emb = np.concatenate([t, t], axis=1)  # [S, 64]
    cos = np.cos(emb).astype(ml_dtypes.bfloat16)
    sin = np.sin(emb).astype(np.float32)
    sin_signed = np.concatenate([-sin[:, :32], sin[:, 32:]], axis=1).astype(ml_dtypes.bfloat16)
    return cos, sin_signed


def _make_in_maps(x, w_qkv, w_proj):
    import ml_dtypes

    x = np.asarray(x, dtype=np.float32)
    w_qkv = np.asarray(w_qkv, dtype=np.float32)
    w_proj = np.asarray(w_proj, dtype=np.float32)

    cos, sin_signed = _rope_tables()
    bf = ml_dtypes.bfloat16

    in_maps = []
    for c in range(NCORES):
        b = c // 4
        g = c % 4
        heads = range(g * HPC, (g + 1) * HPC)
        xTc = np.ascontiguousarray(x[b].T).astype(bf)                    # [DM, S]
        wq = np.concatenate([w_qkv[:, h * HD:(h + 1) * HD] for h in heads], axis=1)
        wk = np.concatenate([w_qkv[:, DM + h * HD:DM + (h + 1) * HD] for h in heads], axis=1)
        wvv = np.concatenate([w_qkv[:, 2 * DM + h * HD:2 * DM + (h + 1) * HD] for h in heads], axis=1)
        wqkc = np.concatenate([wq, wk], axis=1).astype(bf)               # [DM, 512]
        wvv = wvv.astype(bf)                                             # [DM, 256]
        wpl = w_proj[g * 256:(g + 1) * 256, :].astype(bf)                # [256, DM]
        in_maps.append({
            "xT": xTc,
            "wqk": np.ascontiguousarray(wqkc),
            "wv": np.ascontiguousarray(wvv),
            "wp": np.ascontiguousarray(wpl),
            "cos_t": cos,
            "sin_t": sin_signed,
        })
    return in_maps


def kernel(x, w_qkv, w_proj, b_proj):
    from concourse.bass_utils import run_bass_kernel_spmd

    if "nc" not in _cache:
        _cache["nc"] = _build_nc()
    nc = _cache["nc"]

    in_maps = _make_in_maps(x, w_qkv, w_proj)
    res = run_bass_kernel_spmd(nc, in_maps, core_ids=list(range(NCORES)))
    outs = [r["out_partial"].astype(np.float32) for r in res.results]
    b_proj = np.asarray(b_proj, dtype=np.float32)
    full = np.empty((B, S, DM), dtype=np.float32)
    for b in range(B):
        full[b] = (outs[4 * b] + outs[4 * b + 1] + outs[4 * b + 2]
                   + outs[4 * b + 3]) + b_proj
    return full


# revision 44
# speedup vs baseline: 1.4635x; 1.0086x over previous
"""Causal self-attention (B=2, S=2048, dim=1024, 16 heads, RoPE) on 8 trn2 cores.

Sharding: batch x head-group. Core c handles batch c//4 and heads [4*(c%4), 4*(c%4)+4).
QKV is column-parallel, attention embarrassingly parallel per (batch, head), output
projection row-parallel (each core emits a bf16 partial [S, dim] over its heads' 256
attn dims); the host sums the 4 partials per batch and adds b_proj.

Device pipeline per core (matmuls bf16, f32 PSUM accumulation):
  A) QKV: lhsT = x^T tile (host-pretransposed bf16), rhs = w_qkv column slice.
  B) RoPE on Q,K in token-major layout (3 DVE ops using a negative-stride
     half-swap AP and bf16 tables), then ONE XBAR DMA block-transpose per token
     tile moves Q^T/K^T [2h*64, 128] into qkT_all -- no PE transposes, no copies.
  C) Per (head, q-chunk of 512): scores^T = K^T.T @ Q^T chunk -> PSUM pairs,
     exp via ScalarE (scale=1/8 folded; logits O(6) so no max subtraction),
     causal via skipping masked tiles + gpsimd affine_select on diagonal blocks.
     AV reoriented: out[q(128), qs, 65] = P^T-chunk.T @ (V||ones) so the softmax
     denominator lands per-PARTITION: DVE reciprocal + broadcast-mult normalize
     (no gpsimd partition_broadcast). O^T for proj via one XBAR DMA transpose
     per (q-chunk, head-pair).
  D) proj: lhsT = O^T [128, t], rhs = w_proj row-slice; PSUM -> SBUF bf16 via
     DVE; bf16 partial DMA'd out. Startup DMAs ordered wqk -> xT token-chunks
     so the first QKV matmul fires ~6us in.
"""

import sys

sys.path.insert(0, "/opt/trn_rl_repo")

import numpy as np

B = 2
S = 2048
DM = 1024
NH = 16
HD = 64
NCORES = 8
HPC = 4          # heads per core
TT = S // 128    # 16 token tiles
QC = 4           # q-chunks of 512
MAX_WAVELENGTH = 10000.0

_cache = {}


def _build_nc(reps=1):
    import concourse.bass as bass
    import concourse.tile as tile
    import concourse.mybir as mybir
    from concourse import bacc
    from concourse.masks import make_identity

    F32 = mybir.dt.float32
    BF16 = mybir.dt.bfloat16
    Exp = mybir.ActivationFunctionType.Exp

    nc = bacc.Bacc()

    xT = nc.dram_tensor("xT", [DM, S], BF16, kind="ExternalInput")
    wqk = nc.dram_tensor("wqk", [DM, 512], BF16, kind="ExternalInput")
    wv = nc.dram_tensor("wv", [DM, 256], BF16, kind="ExternalInput")
    wp = nc.dram_tensor("wp", [256, DM], BF16, kind="ExternalInput")
    cos_t = nc.dram_tensor("cos_t", [S, HD], BF16, kind="ExternalInput")
    sin_t = nc.dram_tensor("sin_t", [S, HD], BF16, kind="ExternalInput")
    out = nc.dram_tensor("out_partial", [S, DM], BF16, kind="ExternalOutput")

    with tile.TileContext(nc) as tc:
        with tc.tile_pool(name="persist", bufs=1) as persist, \
             tc.tile_pool(name="ropep", bufs=2) as ropep, \
             tc.tile_pool(name="pTp", bufs=2) as pTp, \
             tc.tile_pool(name="onp", bufs=2) as onp, \
             tc.tile_pool(name="smallp", bufs=3) as smallp, \
             tc.tile_pool(name="outp", bufs=6) as outp, \
             tc.tile_pool(name="psQK", bufs=2, space="PSUM") as psQK, \
             tc.tile_pool(name="psV", bufs=1, space="PSUM") as psV, \
             tc.tile_pool(name="psS", bufs=2, space="PSUM") as psS, \
             tc.tile_pool(name="psO", bufs=1, space="PSUM") as psO:
            ident = persist.tile([128, 128], BF16)
            make_identity(nc, ident)

            for _rep in range(reps):
              # --- constant loads, split so the first QKV matmuls can stream
              # as soon as the first wqk/xT half-chunks land (~2us in).
              wqk_sb = persist.tile([128, 8, 512], BF16)
              wqkr = wqk.rearrange("(mc p) c -> p mc c", p=128)
              xT_sb = persist.tile([128, 8, S], BF16)
              xTr = xT.rearrange("(mc p) t -> p mc t", p=128)
              nc.sync.dma_start(wqk_sb[:, 0:4, :], wqkr[:, 0:4, :])
              nc.sync.dma_start(xT_sb[:, 0:4, 0:512], xTr[:, 0:4, 0:512])
              nc.sync.dma_start(wqk_sb[:, 4:8, :], wqkr[:, 4:8, :])
              nc.sync.dma_start(xT_sb[:, 4:8, 0:512], xTr[:, 4:8, 0:512])
              wv_sb = persist.tile([128, 8, 256], BF16)
              nc.sync.dma_start(wv_sb, wv.rearrange("(mc p) c -> p mc c", p=128))
              cos_sb = persist.tile([128, TT, HD], BF16)
              nc.sync.dma_start(cos_sb, cos_t.rearrange("(tt p) d -> p tt d", p=128))
              sin_sb = persist.tile([128, TT, HD], BF16)
              nc.sync.dma_start(sin_sb, sin_t.rearrange("(tt p) d -> p tt d", p=128))
              for tck in range(1, 4):
                  ts = slice(tck * 512, (tck + 1) * 512)
                  nc.sync.dma_start(xT_sb[:, :, ts], xTr[:, :, ts])
              wp_sb = persist.tile([128, 2, DM], BF16)
              nc.sync.dma_start(wp_sb, wp.rearrange("(kc p) n -> p kc n", p=128))

              # PE warm-up: keep TensorE busy during the initial DMAs so the
              # HAM clock gate is at 2.4 GHz when real matmuls arrive.
              warm = psO.tile([128, 128], BF16, tag="o", name="warm")
              for _w in range(20):
                  nc.tensor.transpose(warm, ident, ident)

              # V in token-major with a ones column per head, one tile per
              # token-tile so attention only depends on the tiles it reads
              v_tiles = {}
              for tt in range(TT):
                  v_tiles[tt] = persist.tile([128, HPC, 65], BF16, tag=f"v_{tt}", name=f"v_{tt}")
                  nc.gpsimd.memset(v_tiles[tt][:, :, 64:65], 1.0)
              # roped Q^T/K^T, written by XBAR DMA transpose.
              # cc: 0=Qh01 1=Qh23 2=Kh01 3=Kh23; [p=64*2h, cc, tokens]
              qkT_all = persist.tile([128, 4, S], BF16, tag="qkT", name="qkT")
              # packed O^T for proj lhsT, per q-chunk: [p=2-head dims, pair, 512]
              oT_tiles = {}
              for qi in range(QC):
                  oT_tiles[qi] = persist.tile([128, 2, 512], BF16, tag=f"oT_{qi}", name=f"oT_{qi}")

              def qkv_mms(tt, psqk, mms):
                  ts = slice(tt * 128, (tt + 1) * 128)
                  for mm in mms:
                      nc.tensor.matmul(psqk, xT_sb[:, mm, ts], wqk_sb[:, mm, :],
                                       start=(mm == 0), stop=(mm == 7))

              def v_mms(tt, psv):
                  ts = slice(tt * 128, (tt + 1) * 128)
                  for mm in range(8):
                      nc.tensor.matmul(psv, xT_sb[:, mm, ts], wv_sb[:, mm, :],
                                       start=(mm == 0), stop=(mm == 7))

              def v_copy(tt, psv, on_act=False):
                  # V copyback (cast to bf16); Act for the startup burst
                  # (no exps queued yet, keeps DVE free for the rope chain),
                  # DVE in steady state
                  dst = v_tiles[tt][:, :, 0:64]
                  src = psv.rearrange("p (h d) -> p h d", h=HPC)
                  if on_act:
                      nc.scalar.copy(out=dst, in_=src)
                  else:
                      nc.vector.tensor_copy(out=dst, in_=src)

              def rope_and_transpose(tt, psqk):
                  # RoPE over the 8 (4Q + 4K) 64-wide head blocks of psqk:
                  # t_sin = halfswap(psqk) * sin_signed; t_cos = psqk * cos;
                  # qkro = t_cos + t_sin (all-bf16 SBUF add -> DVE 2x mode)
                  pv4 = psqk.rearrange("p (b h s) -> p b h s", b=8, s=32)
                  swapped = pv4[:, :, ::-1, :]
                  t_sin = ropep.tile([128, 512], BF16, tag="tsin")
                  sv = sin_sb[:, tt, :].rearrange("p (h s) -> p h s", s=32)
                  nc.vector.tensor_tensor(
                      t_sin.rearrange("p (b h s) -> p b h s", b=8, s=32),
                      swapped,
                      sv[:, None, :, :].to_broadcast([128, 8, 2, 32]),
                      mybir.AluOpType.mult)
                  t_cos = ropep.tile([128, 512], BF16, tag="tcos")
                  nc.vector.tensor_tensor(
                      t_cos.rearrange("p (b d) -> p b d", b=8),
                      psqk.rearrange("p (b d) -> p b d", b=8),
                      cos_sb[:, tt, None, :].to_broadcast([128, 8, HD]),
                      mybir.AluOpType.mult)
                  qkro = ropep.tile([128, 512], BF16, tag="qkro")
                  nc.vector.tensor_tensor(qkro, t_cos, t_sin, mybir.AluOpType.add)

                  # one XBAR block transpose: qkT_all[p, cc, t] = qkro[t, cc*128+p]
                  ts = slice(tt * 128, (tt + 1) * 128)
                  nc.sync.dma_start_transpose(qkT_all[:, :, ts], qkro)

              def emit_qkv(tt):
                  psqk = psQK.tile([128, 512], F32, tag="qk",
                                   name=f"psqk_{tt}")
                  qkv_mms(tt, psqk, range(8))
                  psv = psV.tile([128, 256], F32, tag="v", name=f"psv_{tt}")
                  v_mms(tt, psv)
                  rope_and_transpose(tt, psqk)
                  v_copy(tt, psv)

              def qkv_thunks(tt):
                  """QKV for one tile as weighted filler thunks (fine-grained
                  mm units so conservative dealing can place them)."""
                  psqk = psQK.tile([128, 512], F32, tag="qk",
                                   name=f"psqk_f{tt}")
                  psv = psV.tile([128, 256], F32, tag="v", name=f"psv_f{tt}")
                  th = []
                  for mm in range(8):
                      th.append((213, lambda mm=mm: qkv_mms(tt, psqk, [mm])))
                  th.append((50, lambda: rope_and_transpose(tt, psqk)))
                  for mm in range(8):
                      def vmm(mm=mm):
                          ts2 = slice(tt * 128, (tt + 1) * 128)
                          nc.tensor.matmul(psv, xT_sb[:, mm, ts2],
                                           wv_sb[:, mm, :],
                                           start=(mm == 0), stop=(mm == 7))
                      th.append((107, vmm))
                  th.append((50, lambda: v_copy(tt, psv)))
                  return th

              def av_thunks(qc, h, onorm):
                  """AV + normalize for (qc, h) as a list of emission thunks,
                  to be interleaved between the next head's score pairs so PE
                  has work while ScalarE chews through that head's exps."""
                  pT = pT_tiles[h % 2]
                  final = qc == QC - 1 and h == HPC - 1
                  if final:
                      # final head: one psS tile PER qs chain. Dependency
                      # tracking is whole-tile, so a shared pso would give
                      # each chain's first matmul a WAR dep on every prior
                      # chain's normalize reads, serializing the tail.
                      slots = [psS.tile([128, 2, 512], F32, tag="s",
                                        name=f"psf_{qs}")[:, 0, 0:65]
                               for qs in range(4)]
                  else:
                      pso = psO.tile([128, HPC, 65], F32, tag="o",
                                     name=f"pso_{qc}_{h}")
                      slots = [pso[:, qs, :] for qs in range(4)]
                  pbase = (h % 2) * 64
                  thunks = []
                  for qs in range(4):
                      n_kt_qs = 4 * qc + qs + 1
                      for kt in range(n_kt_qs):
                          def mm(qs=qs, kt=kt, n=n_kt_qs):
                              nc.tensor.matmul(
                                  slots[qs],
                                  pT[:, kt, qs * 128:(qs + 1) * 128],
                                  v_tiles[kt][:, h, :],
                                  start=(kt == 0), stop=(kt == n - 1))
                          thunks.append((27, mm))

                  if final:
                      # final head: normalize + transpose per q-subchunk so
                      # each tail proj tile starts as soon as its slice lands;
                      # transposes alternate sync/scalar queues to overlap the
                      # per-issue HWDGE slots
                      def norm_qs(qs):
                          recip = smallp.tile([128, 1], F32, tag="recip",
                                              name=f"recip_{qc}_{h}_{qs}")
                          nc.vector.reciprocal(recip, slots[qs][:, 64:65])
                          nc.vector.tensor_tensor(
                              onorm[:, qs, pbase:pbase + 64],
                              slots[qs][:, 0:64],
                              recip[:, :].to_broadcast([128, 64]),
                              mybir.AluOpType.mult)
                          if qs < 2:
                              eng = nc.sync if qs % 2 == 0 else nc.scalar
                              eng.dma_start_transpose(
                                  oT_tiles[qc][:, h // 2,
                                               qs * 128:(qs + 1) * 128],
                                  onorm[:, qs, :])
                          else:
                              # last two slices: PE transpose + engine copy is
                              # ~1us lower latency than the XBAR DMA path, and
                              # PE is idle at the tail
                              ptr = psO.tile([128, 128], BF16, tag="o",
                                             name=f"ptr_{qs}")
                              nc.tensor.transpose(ptr, onorm[:, qs, :], ident)
                              cp = nc.vector.tensor_copy if qs == 2 \
                                  else nc.scalar.copy
                              cp(out=oT_tiles[qc][:, h // 2,
                                                  qs * 128:(qs + 1) * 128],
                                 in_=ptr)
                      # insert each norm right after its qs chain's last matmul
                      out_thunks = []
                      i = 0
                      for qs in range(4):
                          n_kt_qs = 4 * qc + qs + 1
                          out_thunks.extend(thunks[i:i + n_kt_qs])
                          i += n_kt_qs
                          out_thunks.append((50, lambda qs=qs: norm_qs(qs)))
                      return out_thunks

                  def norm():
                      recip = smallp.tile([128, 4], F32, tag="recip",
                                          name=f"recip_{qc}_{h}")
                      nc.vector.reciprocal(recip, pso[:, :, 64])
                      nc.vector.tensor_tensor(
                          onorm[:, :, pbase:pbase + 64],
                          pso[:, :, 0:64],
                          recip[:, :, None].to_broadcast([128, 4, 64]),
                          mybir.AluOpType.mult)
                      if h % 2 == 1:  # head pair complete -> O^T via XBAR
                          nc.sync.dma_start_transpose(
                              oT_tiles[qc][:, h // 2, :].rearrange(
                                  "p (a b) -> p a b", a=4),
                              onorm)
                  thunks.append((50, norm))
                  return thunks

              def emit_attn(qc, h, fillers):
                  """Score pairs + exps for (qc, h), with filler thunks (AV of
                  the previous head, proj tiles) interleaved between pairs."""
                  n_kt = 4 * (qc + 1)
                  n_pairs = n_kt // 2
                  pbase = (h % 2) * 64
                  qT = qkT_all[pbase:pbase + 64, h // 2, qc * 512:(qc + 1) * 512]
                  pT = pTp.tile([128, TT, 512], BF16, tag="pT",
                                name=f"pT_{qc}_{h}")
                  pT_tiles[h % 2] = pT
                  # deal filler thunks between pairs weighted by their PE cost
                  # so each inter-pair slot gets roughly equal fill time
                  total_cost = sum(c for c, _ in fillers)
                  done_cost = 0.0
                  fi = 0
                  for kp in range(n_pairs):
                      ps2 = psS.tile([128, 2, 512], F32, tag="s",
                                     name=f"s_{qc}_{h}_{kp}")
                      for ki in range(2):
                          kt = kp * 2 + ki
                          j = kt - 4 * qc  # >=0 on diagonal-crossing tiles
                          cs = max(0, j * 128)
                          kT = qkT_all[pbase:pbase + 64, 2 + h // 2,
                                       kt * 128:(kt + 1) * 128]
                          nc.tensor.matmul(
                              ps2[:, ki, cs:512],
                              kT,
                              qT[:, cs:512],
                              start=True, stop=True)
                      if kp * 2 < 4 * qc:  # both tiles full: single big exp
                          nc.scalar.activation(
                              out=pT[:, kp * 2:kp * 2 + 2, :],
                              in_=ps2,
                              func=Exp, scale=0.125)
                      else:
                          for ki in range(2):
                              kt = kp * 2 + ki
                              j = kt - 4 * qc
                              cs = j * 128
                              nc.scalar.activation(out=pT[:, kt, cs:512],
                                                   in_=ps2[:, ki, cs:512],
                                                   func=Exp, scale=0.125)
                      for ki in range(2):
                          kt = kp * 2 + ki
                          j = kt - 4 * qc
                          if j >= 0:
                              blk = slice(j * 128, (j + 1) * 128)
                              nc.gpsimd.affine_select(
                                  out=pT[:, kt, blk], in_=pT[:, kt, blk],
                                  pattern=[[1, 128]], channel_multiplier=-1,
                                  base=0, compare_op=mybir.AluOpType.is_ge,
                                  fill=0.0)
                      # conservative fill: never overshoot the slot quota, so
                      # score pairs are not delayed past psS readiness and the
                      # ScalarE exp cadence (the binding rate late in the
                      # kernel) is preserved; leftovers run after the loop
                      quota = total_cost * (kp + 1) / n_pairs
                      while (fi < len(fillers)
                             and done_cost + fillers[fi][0] <= quota):
                          done_cost += fillers[fi][0]
                          fillers[fi][1]()
                          fi += 1
                  # leftovers are NOT flushed here: emitting them now would
                  # sit between this head's last pair and the next head's
                  # first pair, delaying the exp cadence; the caller carries
                  # them into the next head's filler list instead
                  return fillers[fi:]

              def proj_half(tt, nn, osb, tail):
                  # proj PSUM comes from the psQK pool (shared with the QKV
                  # psqk rotation) so psS stays dedicated to score pairs.
                  # Tail tiles split copies across Act (idle then) and DVE.
                  ts = slice(tt * 128, (tt + 1) * 128)
                  tl = oT_tiles[tt // 4]
                  tsl = slice((tt % 4) * 128, (tt % 4 + 1) * 128)
                  ns = slice(nn * 512, (nn + 1) * 512)
                  pj = psQK.tile([128, 512], F32, tag="qk",
                                 name=f"pj_{tt}_{nn}")
                  nc.tensor.matmul(pj, tl[:, 0, tsl], wp_sb[:, 0, ns],
                                   start=True, stop=False)
                  nc.tensor.matmul(pj, tl[:, 1, tsl], wp_sb[:, 1, ns],
                                   start=False, stop=True)
                  if tail and nn == 0:
                      nc.scalar.copy(out=osb[:, ns], in_=pj)
                  else:
                      nc.vector.tensor_copy(out=osb[:, ns], in_=pj)
                  if nn == 1:
                      nc.sync.dma_start(out[ts, :], osb)

              def proj_thunks(tt, tail=False):
                  osb = outp.tile([128, DM], BF16, tag="osb",
                                  name=f"osb_{tt}")
                  return [(430, lambda nn=nn: proj_half(tt, nn, osb, tail))
                          for nn in range(2)]

              def emit_proj(tt, tail=False):
                  for _, th in proj_thunks(tt, tail):
                      th()

              # software-pipelined emission: QKV for q-chunk 0 up front; then
              # per (qc, h): one QKV tile of qc+1 ahead of the head's scores,
              # with the previous head's AV+normalize and scheduled proj tiles
              # interleaved between score pairs (PE filler while ScalarE exps).
              # all deferrable proj work lands in qc3's cycles, where ScalarE's
              # exp hump would otherwise leave PE idle; QKV fillers finish by
              # h2 so the next chunk's last rope+transpose beats scores(qc+1,h0)
              proj_sched = {(3, 0): [0, 1, 2], (3, 1): [3, 4, 5],
                            (3, 2): [6, 7, 8], (3, 3): [9, 10, 11]}
              qkv_sched = {0: [0], 1: [1], 2: [2]}
              # startup: stream tiles 0/1 mm-major in wqk/xT half-chunk order
              # so PE follows the arriving DMA halves instead of waiting for
              # the full 2MB; tiles 2/3 go through the normal path.
              psqk_s = {tt: psQK.tile([128, 512], F32, tag="qk",
                                      name=f"psqk_s{tt}")
                        for tt in (0, 1)}
              for tt in (0, 1):
                  qkv_mms(tt, psqk_s[tt], range(0, 4))
              for tt in (0, 1):
                  qkv_mms(tt, psqk_s[tt], range(4, 8))
              rope_and_transpose(0, psqk_s[0])
              psv_s0 = psV.tile([128, 256], F32, tag="v", name="psv_s0")
              v_mms(0, psv_s0)
              rope_and_transpose(1, psqk_s[1])
              v_copy(0, psv_s0, on_act=True)
              psv_s1 = psV.tile([128, 256], F32, tag="v", name="psv_s1")
              v_mms(1, psv_s1)
              v_copy(1, psv_s1, on_act=True)
              for tt in (2, 3):
                  psqk = psQK.tile([128, 512], F32, tag="qk",
                                   name=f"psqk_s{tt}")
                  qkv_mms(tt, psqk, range(8))
                  psv = psV.tile([128, 256], F32, tag="v", name=f"psv_s{tt}")
                  v_mms(tt, psv)
                  rope_and_transpose(tt, psqk)
                  v_copy(tt, psv, on_act=True)
              pT_tiles = {}
              av_prev = []
              carry = []
              for qc in range(QC):
                  onorms = [onp.tile([128, 4, 128], BF16, tag="on",
                                     name=f"on_{qc}_{i}")
                            for i in range(2)]
                  for h in range(HPC):
                      if qc < QC - 1:
                          for dt in qkv_sched.get(h, []):
                              emit_qkv(4 * (qc + 1) + dt)
                      fillers = carry + list(av_prev)
                      if h == 3 and qc < QC - 1:
                          # 4th QKV tile of the next chunk rides as fillers so
                          # its rope+transpose complete before scores(qc+1,h0)
                          fillers.extend(qkv_thunks(4 * (qc + 1) + 3))
                      for tt in proj_sched.get((qc, h), []):
                          fillers.extend(proj_thunks(tt))
                      carry = emit_attn(qc, h, fillers)
                      av_prev = av_thunks(qc, h, onorms[h // 2])
              for _, th in carry + av_prev:
                  th()
              # tail proj: per-tile DMAs (lowest last-byte latency), copies
              # split across Act and DVE
              for tt in range(12, 16):
                  emit_proj(tt, tail=True)

    nc.finalize()
    return nc


def _rope_tables():
    import ml_dtypes
    inv_freq = 1.0 / (MAX_WAVELENGTH ** (np.arange(0, HD, 2, dtype=np.float32) / HD))
    t = np.arange(S, dtype=np.float32)[:, None] * inv_freq[None, :]  # [S, 32]
    emb = np.concatenate([t, t], axis=1)  # [S, 64]
    cos = np.cos(emb).astype(ml_dtypes.bfloat16)
    sin = np.sin(emb).astype(np.float32)
    sin_signed = np.concatenate([-sin[:, :32], sin[:, 32:]], axis=1).astype(ml_dtypes.bfloat16)
    return cos, sin_signed


def _make_in_maps(x, w_qkv, w_proj):
    import ml_dtypes

    x = np.asarray(x, dtype=np.float32)
    w_qkv = np.asarray(w_qkv, dtype=np.float32)
    w_proj = np.asarray(w_proj, dtype=np.float32)

    cos, sin_signed = _rope_tables()
    bf = ml_dtypes.bfloat16

    in_maps = []
    for c in range(NCORES):
        b = c // 4
        g = c % 4
        heads = range(g * HPC, (g + 1) * HPC)
        xTc = np.ascontiguousarray(x[b].T).astype(bf)                    # [DM, S]
        wq = np.concatenate([w_qkv[:, h * HD:(h + 1) * HD] for h in heads], axis=1)
        wk = np.concatenate([w_qkv[:, DM + h * HD:DM + (h + 1) * HD] for h in heads], axis=1)
        wvv = np.concatenate([w_qkv[:, 2 * DM + h * HD:2 * DM + (h + 1) * HD] for h in heads], axis=1)
        wqkc = np.concatenate([wq, wk], axis=1).astype(bf)               # [DM, 512]
        wvv = wvv.astype(bf)                                             # [DM, 256]
        wpl = w_proj[g * 256:(g + 1) * 256, :].astype(bf)                # [256, DM]
        in_maps.append({
            "xT": xTc,
            "wqk": np.ascontiguousarray(wqkc),
            "wv": np.ascontiguousarray(wvv),
            "wp": np.ascontiguousarray(wpl),
            "cos_t": cos,
            "sin_t": sin_signed,
        })
    return in_maps


def kernel(x, w_qkv, w_proj, b_proj):
    from concourse.bass_utils import run_bass_kernel_spmd

    if "nc" not in _cache:
        _cache["nc"] = _build_nc()
    nc = _cache["nc"]

    in_maps = _make_in_maps(x, w_qkv, w_proj)
    res = run_bass_kernel_spmd(nc, in_maps, core_ids=list(range(NCORES)))
    outs = [r["out_partial"].astype(np.float32) for r in res.results]
    b_proj = np.asarray(b_proj, dtype=np.float32)
    full = np.empty((B, S, DM), dtype=np.float32)
    for b in range(B):
        full[b] = (outs[4 * b] + outs[4 * b + 1] + outs[4 * b + 2]
                   + outs[4 * b + 3]) + b_proj
    return full


# revision 51
# speedup vs baseline: 1.5334x; 1.0478x over previous
"""Causal self-attention (B=2, S=2048, dim=1024, 16 heads, RoPE) on 8 trn2 cores.

Sharding: batch x head-group. Core c handles batch c//4 and heads [4*(c%4), 4*(c%4)+4).
QKV is column-parallel, attention embarrassingly parallel per (batch, head), output
projection row-parallel (each core emits a bf16 partial [S, dim] over its heads' 256
attn dims); the host sums the 4 partials per batch and adds b_proj.

Device pipeline per core (matmuls bf16, f32 PSUM accumulation):
  A) QKV: lhsT = x^T tile (host-pretransposed bf16), rhs = w_qkv column slice.
  B) RoPE on Q,K in token-major layout (3 DVE ops using a negative-stride
     half-swap AP and bf16 tables), then ONE XBAR DMA block-transpose per token
     tile moves Q^T/K^T [2h*64, 128] into qkT_all -- no PE transposes, no copies.
  C) Per (head, q-chunk of 512): scores^T = K^T.T @ Q^T chunk -> PSUM pairs,
     exp via ScalarE (scale=1/8 folded; logits O(6) so no max subtraction),
     causal via skipping masked tiles + gpsimd affine_select on diagonal blocks.
     AV reoriented: out[q(128), qs, 65] = P^T-chunk.T @ (V||ones) so the softmax
     denominator lands per-PARTITION: DVE reciprocal + broadcast-mult normalize
     (no gpsimd partition_broadcast). O^T for proj via one XBAR DMA transpose
     per (q-chunk, head-pair).
  D) proj: lhsT = O^T [128, t], rhs = w_proj row-slice; PSUM -> SBUF bf16 via
     DVE; bf16 partial DMA'd out. Startup DMAs ordered wqk -> xT token-chunks
     so the first QKV matmul fires ~6us in.
"""

import sys

sys.path.insert(0, "/opt/trn_rl_repo")

import numpy as np

B = 2
S = 2048
DM = 1024
NH = 16
HD = 64
NCORES = 8
HPC = 4          # heads per core
TT = S // 128    # 16 token tiles
QC = 4           # q-chunks of 512
MAX_WAVELENGTH = 10000.0

_cache = {}


def _build_nc(reps=1):
    import concourse.bass as bass
    import concourse.tile as tile
    import concourse.mybir as mybir
    from concourse import bacc
    from concourse.masks import make_identity

    F32 = mybir.dt.float32
    BF16 = mybir.dt.bfloat16
    Exp = mybir.ActivationFunctionType.Exp

    nc = bacc.Bacc()

    xT = nc.dram_tensor("xT", [DM, S], BF16, kind="ExternalInput")
    wqk = nc.dram_tensor("wqk", [DM, 512], BF16, kind="ExternalInput")
    wv = nc.dram_tensor("wv", [DM, 256], BF16, kind="ExternalInput")
    wp = nc.dram_tensor("wp", [256, DM], BF16, kind="ExternalInput")
    cos_t = nc.dram_tensor("cos_t", [S, HD], BF16, kind="ExternalInput")
    sin_t = nc.dram_tensor("sin_t", [S, HD], BF16, kind="ExternalInput")
    out = nc.dram_tensor("out_partial", [S, DM], BF16, kind="ExternalOutput")

    with tile.TileContext(nc) as tc:
        with tc.tile_pool(name="persist", bufs=1) as persist, \
             tc.tile_pool(name="ropep", bufs=6) as ropep, \
             tc.tile_pool(name="pTp", bufs=3) as pTp, \
             tc.tile_pool(name="onp", bufs=4) as onp, \
             tc.tile_pool(name="smallp", bufs=6) as smallp, \
             tc.tile_pool(name="outp", bufs=6) as outp, \
             tc.tile_pool(name="psQK", bufs=2, space="PSUM") as psQK, \
             tc.tile_pool(name="psV", bufs=1, space="PSUM") as psV, \
             tc.tile_pool(name="psS", bufs=2, space="PSUM") as psS, \
             tc.tile_pool(name="psO", bufs=1, space="PSUM") as psO:
            ident = persist.tile([128, 128], BF16)
            make_identity(nc, ident)

            for _rep in range(reps):
              # --- constant loads, split so the first QKV matmuls can stream
              # as soon as the first wqk/xT half-chunks land (~2us in).
              wqk_sb = persist.tile([128, 8, 512], BF16)
              wqkr = wqk.rearrange("(mc p) c -> p mc c", p=128)
              xT_sb = persist.tile([128, 8, S], BF16)
              xTr = xT.rearrange("(mc p) t -> p mc t", p=128)
              nc.sync.dma_start(wqk_sb[:, 0:4, :], wqkr[:, 0:4, :])
              nc.sync.dma_start(xT_sb[:, 0:4, 0:512], xTr[:, 0:4, 0:512])
              nc.sync.dma_start(wqk_sb[:, 4:8, :], wqkr[:, 4:8, :])
              nc.sync.dma_start(xT_sb[:, 4:8, 0:512], xTr[:, 4:8, 0:512])
              wv_sb = persist.tile([128, 8, 256], BF16)
              nc.sync.dma_start(wv_sb, wv.rearrange("(mc p) c -> p mc c", p=128))
              cos_sb = persist.tile([128, TT, HD], BF16)
              nc.sync.dma_start(cos_sb, cos_t.rearrange("(tt p) d -> p tt d", p=128))
              sin_sb = persist.tile([128, TT, HD], BF16)
              nc.sync.dma_start(sin_sb, sin_t.rearrange("(tt p) d -> p tt d", p=128))
              for tck in range(1, 4):
                  ts = slice(tck * 512, (tck + 1) * 512)
                  nc.sync.dma_start(xT_sb[:, :, ts], xTr[:, :, ts])
              wp_sb = persist.tile([128, 2, DM], BF16)
              nc.sync.dma_start(wp_sb, wp.rearrange("(kc p) n -> p kc n", p=128))

              # PE warm-up: keep TensorE busy during the initial DMAs so the
              # HAM clock gate is at 2.4 GHz when real matmuls arrive.
              warm = psO.tile([128, 128], BF16, tag="o", name="warm")
              for _w in range(20):
                  nc.tensor.transpose(warm, ident, ident)

              # V in token-major with a ones column per head, one tile per
              # token-tile so attention only depends on the tiles it reads
              v_tiles = {}
              for tt in range(TT):
                  v_tiles[tt] = persist.tile([128, HPC, 65], BF16, tag=f"v_{tt}", name=f"v_{tt}")
                  nc.gpsimd.memset(v_tiles[tt][:, :, 64:65], 1.0)
              # roped Q^T/K^T, written by XBAR DMA transpose.
              # cc: 0=Qh01 1=Qh23 2=Kh01 3=Kh23; [p=64*2h, cc, tokens]
              qkT_all = persist.tile([128, 4, S], BF16, tag="qkT", name="qkT")
              # packed O^T for proj lhsT, per q-chunk: [p=2-head dims, pair, 512]
              oT_tiles = {}
              for qi in range(QC):
                  oT_tiles[qi] = persist.tile([128, 2, 512], BF16, tag=f"oT_{qi}", name=f"oT_{qi}")

              def qkv_mms(tt, psqk, mms):
                  ts = slice(tt * 128, (tt + 1) * 128)
                  for mm in mms:
                      nc.tensor.matmul(psqk, xT_sb[:, mm, ts], wqk_sb[:, mm, :],
                                       start=(mm == 0), stop=(mm == 7))

              def v_mms(tt, psv):
                  ts = slice(tt * 128, (tt + 1) * 128)
                  for mm in range(8):
                      nc.tensor.matmul(psv, xT_sb[:, mm, ts], wv_sb[:, mm, :],
                                       start=(mm == 0), stop=(mm == 7))

              def v_copy(tt, psv, on_act=False):
                  # V copyback (cast to bf16); Act for the startup burst
                  # (no exps queued yet, keeps DVE free for the rope chain),
                  # DVE in steady state
                  dst = v_tiles[tt][:, :, 0:64]
                  src = psv.rearrange("p (h d) -> p h d", h=HPC)
                  if on_act:
                      nc.scalar.copy(out=dst, in_=src)
                  else:
                      nc.vector.tensor_copy(out=dst, in_=src)

              def rope_and_transpose(tt, psqk):
                  # RoPE over the 8 (4Q + 4K) 64-wide head blocks of psqk:
                  # t_sin = halfswap(psqk) * sin_signed; t_cos = psqk * cos;
                  # qkro = t_cos + t_sin (all-bf16 SBUF add -> DVE 2x mode)
                  pv4 = psqk.rearrange("p (b h s) -> p b h s", b=8, s=32)
                  swapped = pv4[:, :, ::-1, :]
                  t_sin = ropep.tile([128, 512], BF16, tag="tsin")
                  sv = sin_sb[:, tt, :].rearrange("p (h s) -> p h s", s=32)
                  nc.vector.tensor_tensor(
                      t_sin.rearrange("p (b h s) -> p b h s", b=8, s=32),
                      swapped,
                      sv[:, None, :, :].to_broadcast([128, 8, 2, 32]),
                      mybir.AluOpType.mult)
                  t_cos = ropep.tile([128, 512], BF16, tag="tcos")
                  nc.vector.tensor_tensor(
                      t_cos.rearrange("p (b d) -> p b d", b=8),
                      psqk.rearrange("p (b d) -> p b d", b=8),
                      cos_sb[:, tt, None, :].to_broadcast([128, 8, HD]),
                      mybir.AluOpType.mult)
                  qkro = ropep.tile([128, 512], BF16, tag="qkro")
                  nc.vector.tensor_tensor(qkro, t_cos, t_sin, mybir.AluOpType.add)

                  # one XBAR block transpose: qkT_all[p, cc, t] = qkro[t, cc*128+p]
                  ts = slice(tt * 128, (tt + 1) * 128)
                  nc.sync.dma_start_transpose(qkT_all[:, :, ts], qkro)

              def emit_qkv(tt):
                  psqk = psQK.tile([128, 512], F32, tag="qk",
                                   name=f"psqk_{tt}")
                  qkv_mms(tt, psqk, range(8))
                  psv = psV.tile([128, 256], F32, tag="v", name=f"psv_{tt}")
                  v_mms(tt, psv)
                  rope_and_transpose(tt, psqk)
                  v_copy(tt, psv, on_act=tt < 8)

              def qkv_thunks(tt):
                  """QKV for one tile as weighted filler thunks (fine-grained
                  mm units so conservative dealing can place them)."""
                  psqk = psQK.tile([128, 512], F32, tag="qk",
                                   name=f"psqk_f{tt}")
                  psv = psV.tile([128, 256], F32, tag="v", name=f"psv_f{tt}")
                  th = []
                  for mm in range(8):
                      th.append((213, lambda mm=mm: qkv_mms(tt, psqk, [mm])))
                  th.append((50, lambda: rope_and_transpose(tt, psqk)))
                  for mm in range(8):
                      def vmm(mm=mm):
                          ts2 = slice(tt * 128, (tt + 1) * 128)
                          nc.tensor.matmul(psv, xT_sb[:, mm, ts2],
                                           wv_sb[:, mm, :],
                                           start=(mm == 0), stop=(mm == 7))
                      th.append((107, vmm))
                  th.append((50, lambda: v_copy(tt, psv)))
                  return th

              def av_thunks(qc, h, onorm):
                  """AV + normalize for (qc, h) as a list of emission thunks,
                  to be interleaved between the next head's score pairs so PE
                  has work while ScalarE chews through that head's exps."""
                  pT = pT_tiles[h % 2]
                  final = qc == QC - 1 and h == HPC - 1
                  if final:
                      # final head: one psS tile PER qs chain. Dependency
                      # tracking is whole-tile, so a shared pso would give
                      # each chain's first matmul a WAR dep on every prior
                      # chain's normalize reads, serializing the tail.
                      slots = [psS.tile([128, 2, 512], F32, tag="s",
                                        name=f"psf_{qs}")[:, 0, 0:65]
                               for qs in range(4)]
                  else:
                      pso = psO.tile([128, HPC, 65], F32, tag="o",
                                     name=f"pso_{qc}_{h}")
                      slots = [pso[:, qs, :] for qs in range(4)]
                  pbase = (h % 2) * 64
                  thunks = []
                  for qs in range(4):
                      n_kt_qs = 4 * qc + qs + 1
                      for kt in range(n_kt_qs):
                          def mm(qs=qs, kt=kt, n=n_kt_qs):
                              nc.tensor.matmul(
                                  slots[qs],
                                  pT[:, kt, qs * 128:(qs + 1) * 128],
                                  v_tiles[kt][:, h, :],
                                  start=(kt == 0), stop=(kt == n - 1))
                          thunks.append((27, mm))

                  if final:
                      # final head: normalize + transpose per q-subchunk so
                      # each tail proj tile starts as soon as its slice lands;
                      # transposes alternate sync/scalar queues to overlap the
                      # per-issue HWDGE slots
                      def norm_qs(qs):
                          recip = smallp.tile([128, 1], F32, tag="recip",
                                              name=f"recip_{qc}_{h}_{qs}")
                          nc.vector.reciprocal(recip, slots[qs][:, 64:65])
                          nc.vector.tensor_tensor(
                              onorm[:, qs, pbase:pbase + 64],
                              slots[qs][:, 0:64],
                              recip[:, :].to_broadcast([128, 64]),
                              mybir.AluOpType.mult)
                          if qs < 2:
                              eng = nc.sync if qs % 2 == 0 else nc.scalar
                              eng.dma_start_transpose(
                                  oT_tiles[qc][:, h // 2,
                                               qs * 128:(qs + 1) * 128],
                                  onorm[:, qs, :])
                          else:
                              # last two slices: PE transpose + engine copy is
                              # ~1us lower latency than the XBAR DMA path, and
                              # PE is idle at the tail
                              ptr = psO.tile([128, 128], BF16, tag="o",
                                             name=f"ptr_{qs}")
                              nc.tensor.transpose(ptr, onorm[:, qs, :], ident)
                              cp = nc.vector.tensor_copy if qs == 2 \
                                  else nc.scalar.copy
                              cp(out=oT_tiles[qc][:, h // 2,
                                                  qs * 128:(qs + 1) * 128],
                                 in_=ptr)
                      # insert each norm right after its qs chain's last matmul
                      out_thunks = []
                      i = 0
                      for qs in range(4):
                          n_kt_qs = 4 * qc + qs + 1
                          out_thunks.extend(thunks[i:i + n_kt_qs])
                          i += n_kt_qs
                          out_thunks.append((50, lambda qs=qs: norm_qs(qs)))
                      return out_thunks

                  def norm():
                      recip = smallp.tile([128, 4], F32, tag="recip",
                                          name=f"recip_{qc}_{h}")
                      nc.vector.reciprocal(recip, pso[:, :, 64])
                      nc.vector.tensor_tensor(
                          onorm[:, :, pbase:pbase + 64],
                          pso[:, :, 0:64],
                          recip[:, :, None].to_broadcast([128, 4, 64]),
                          mybir.AluOpType.mult)
                      if h % 2 == 1:  # head pair complete -> O^T via XBAR
                          nc.sync.dma_start_transpose(
                              oT_tiles[qc][:, h // 2, :].rearrange(
                                  "p (a b) -> p a b", a=4),
                              onorm)
                  thunks.append((50, norm))
                  return thunks

              def emit_attn(qc, h, fillers):
                  """Score pairs + exps for (qc, h), with filler thunks (AV of
                  the previous head, proj tiles) interleaved between pairs."""
                  n_kt = 4 * (qc + 1)
                  n_pairs = n_kt // 2
                  pbase = (h % 2) * 64
                  qT = qkT_all[pbase:pbase + 64, h // 2, qc * 512:(qc + 1) * 512]
                  pT = pTp.tile([128, TT, 512], BF16, tag="pT",
                                name=f"pT_{qc}_{h}")
                  pT_tiles[h % 2] = pT
                  # deal filler thunks between pairs weighted by their PE cost
                  # so each inter-pair slot gets roughly equal fill time
                  total_cost = sum(c for c, _ in fillers)
                  done_cost = 0.0
                  fi = 0
                  for kp in range(n_pairs):
                      ps2 = psS.tile([128, 2, 512], F32, tag="s",
                                     name=f"s_{qc}_{h}_{kp}")
                      for ki in range(2):
                          kt = kp * 2 + ki
                          j = kt - 4 * qc  # >=0 on diagonal-crossing tiles
                          cs = max(0, j * 128)
                          kT = qkT_all[pbase:pbase + 64, 2 + h // 2,
                                       kt * 128:(kt + 1) * 128]
                          nc.tensor.matmul(
                              ps2[:, ki, cs:512],
                              kT,
                              qT[:, cs:512],
                              start=True, stop=True)
                      if kp * 2 < 4 * qc:  # both tiles full: single big exp
                          nc.scalar.activation(
                              out=pT[:, kp * 2:kp * 2 + 2, :],
                              in_=ps2,
                              func=Exp, scale=0.125)
                      elif kp * 2 == 4 * qc:
                          # first diagonal pair (j=0,1): one exp over both
                          # tiles. Tile j=1's cols [0:128] are unwritten PSUM
                          # exp'd to garbage, but no AV chain ever reads them
                          # (chain qs only reads column block qs of tile
                          # 4qc+j, and j=1 > qs=0), so this is safe and saves
                          # an Act instruction's fixed cost per head.
                          nc.scalar.activation(
                              out=pT[:, kp * 2:kp * 2 + 2, :],
                              in_=ps2,
                              func=Exp, scale=0.125)
                      else:
                          for ki in range(2):
                              kt = kp * 2 + ki
                              j = kt - 4 * qc
                              cs = j * 128
                              nc.scalar.activation(out=pT[:, kt, cs:512],
                                                   in_=ps2[:, ki, cs:512],
                                                   func=Exp, scale=0.125)
                      for ki in range(2):
                          kt = kp * 2 + ki
                          j = kt - 4 * qc
                          if j >= 0:
                              blk = slice(j * 128, (j + 1) * 128)
                              nc.gpsimd.affine_select(
                                  out=pT[:, kt, blk], in_=pT[:, kt, blk],
                                  pattern=[[1, 128]], channel_multiplier=-1,
                                  base=0, compare_op=mybir.AluOpType.is_ge,
                                  fill=0.0)
                      # conservative fill: never overshoot the slot quota, so
                      # score pairs are not delayed past psS readiness and the
                      # ScalarE exp cadence (the binding rate late in the
                      # kernel) is preserved; leftovers run after the loop
                      quota = total_cost * (kp + 1) / n_pairs
                      while (fi < len(fillers)
                             and done_cost + fillers[fi][0] <= quota):
                          done_cost += fillers[fi][0]
                          fillers[fi][1]()
                          fi += 1
                  # leftovers are NOT flushed here: emitting them now would
                  # sit between this head's last pair and the next head's
                  # first pair, delaying the exp cadence; the caller carries
                  # them into the next head's filler list instead
                  return fillers[fi:]

              def proj_half(tt, nn, osb, tail):
                  # proj PSUM comes from the psQK pool (shared with the QKV
                  # psqk rotation) so psS stays dedicated to score pairs.
                  # Tail tiles split copies across Act (idle then) and DVE.
                  ts = slice(tt * 128, (tt + 1) * 128)
                  tl = oT_tiles[tt // 4]
                  tsl = slice((tt % 4) * 128, (tt % 4 + 1) * 128)
                  ns = slice(nn * 512, (nn + 1) * 512)
                  pj = psQK.tile([128, 512], F32, tag="qk",
                                 name=f"pj_{tt}_{nn}")
                  nc.tensor.matmul(pj, tl[:, 0, tsl], wp_sb[:, 0, ns],
                                   start=True, stop=False)
                  nc.tensor.matmul(pj, tl[:, 1, tsl], wp_sb[:, 1, ns],
                                   start=False, stop=True)
                  if tail and nn == 0:
                      nc.scalar.copy(out=osb[:, ns], in_=pj)
                  else:
                      nc.vector.tensor_copy(out=osb[:, ns], in_=pj)
                  if nn == 1:
                      nc.sync.dma_start(out[ts, :], osb)

              def proj_thunks(tt, tail=False):
                  osb = outp.tile([128, DM], BF16, tag="osb",
                                  name=f"osb_{tt}")
                  return [(430, lambda nn=nn: proj_half(tt, nn, osb, tail))
                          for nn in range(2)]

              def emit_proj(tt, tail=False):
                  for _, th in proj_thunks(tt, tail):
                      th()

              # software-pipelined emission: QKV for q-chunk 0 up front; then
              # per (qc, h): one QKV tile of qc+1 ahead of the head's scores,
              # with the previous head's AV+normalize and scheduled proj tiles
              # interleaved between score pairs (PE filler while ScalarE exps).
              # all deferrable proj work lands in qc3's cycles, where ScalarE's
              # exp hump would otherwise leave PE idle; QKV fillers finish by
              # h2 so the next chunk's last rope+transpose beats scores(qc+1,h0)
              proj_sched = {(3, 0): [0, 1, 2], (3, 1): [3, 4, 5],
                            (3, 2): [6, 7, 8], (3, 3): [9, 10, 11]}
              qkv_sched = {0: [0], 1: [1], 2: [2]}
              # startup: stream tiles 0/1 mm-major in wqk/xT half-chunk order
              # so PE follows the arriving DMA halves instead of waiting for
              # the full 2MB; tiles 2/3 go through the normal path.
              psqk_s = {tt: psQK.tile([128, 512], F32, tag="qk",
                                      name=f"psqk_s{tt}")
                        for tt in (0, 1)}
              for tt in (0, 1):
                  qkv_mms(tt, psqk_s[tt], range(0, 4))
              for tt in (0, 1):
                  qkv_mms(tt, psqk_s[tt], range(4, 8))
              rope_and_transpose(0, psqk_s[0])
              psv_s0 = psV.tile([128, 256], F32, tag="v", name="psv_s0")
              v_mms(0, psv_s0)
              rope_and_transpose(1, psqk_s[1])
              v_copy(0, psv_s0, on_act=True)
              psv_s1 = psV.tile([128, 256], F32, tag="v", name="psv_s1")
              v_mms(1, psv_s1)
              v_copy(1, psv_s1, on_act=True)
              for tt in (2, 3):
                  # psS is idle until the first scores: borrow it so the
                  # psqk rotation is 4 deep while the startup tiles stream
                  psqk = psS.tile([128, 2, 512], F32, tag="s",
                                  name=f"psqk_s{tt}")[:, 0, :]
                  qkv_mms(tt, psqk, range(8))
                  psv = psV.tile([128, 256], F32, tag="v", name=f"psv_s{tt}")
                  v_mms(tt, psv)
                  rope_and_transpose(tt, psqk)
                  v_copy(tt, psv, on_act=True)
              pT_tiles = {}
              av_prev = []
              carry = []
              for qc in range(QC):
                  onorms = [onp.tile([128, 4, 128], BF16, tag="on",
                                     name=f"on_{qc}_{i}")
                            for i in range(2)]
                  for h in range(HPC):
                      if qc < QC - 1:
                          for dt in qkv_sched.get(h, []):
                              emit_qkv(4 * (qc + 1) + dt)
                      fillers = carry + list(av_prev)
                      if h == 3 and qc < QC - 1:
                          # 4th QKV tile of the next chunk rides as fillers so
                          # its rope+transpose complete before scores(qc+1,h0)
                          fillers.extend(qkv_thunks(4 * (qc + 1) + 3))
                      for tt in proj_sched.get((qc, h), []):
                          fillers.extend(proj_thunks(tt))
                      carry = emit_attn(qc, h, fillers)
                      av_prev = av_thunks(qc, h, onorms[h // 2])
              for _, th in carry + av_prev:
                  th()
              # tail proj: per-tile DMAs (lowest last-byte latency), copies
              # split across Act and DVE
              for tt in range(12, 16):
                  emit_proj(tt, tail=True)

    nc.finalize()
    return nc


def _rope_tables():
    import ml_dtypes
    inv_freq = 1.0 / (MAX_WAVELENGTH ** (np.arange(0, HD, 2, dtype=np.float32) / HD))
    t = np.arange(S, dtype=np.float32)[:, None] * inv_freq[None, :]  # [S, 32]
    emb = np.concatenate([t, t], axis=1)  # [S, 64]
    cos = np.cos(emb).astype(ml_dtypes.bfloat16)
    sin = np.sin(emb).astype(np.float32)
    sin_signed = np.concatenate([-sin[:, :32], sin[:, 32:]], axis=1).astype(ml_dtypes.bfloat16)
    return cos, sin_signed


def _make_in_maps(x, w_qkv, w_proj):
    import ml_dtypes

    x = np.asarray(x, dtype=np.float32)
    w_qkv = np.asarray(w_qkv, dtype=np.float32)
    w_proj = np.asarray(w_proj, dtype=np.float32)

    cos, sin_signed = _rope_tables()
    bf = ml_dtypes.bfloat16

    in_maps = []
    for c in range(NCORES):
        b = c // 4
        g = c % 4
        heads = range(g * HPC, (g + 1) * HPC)
        xTc = np.ascontiguousarray(x[b].T).astype(bf)                    # [DM, S]
        wq = np.concatenate([w_qkv[:, h * HD:(h + 1) * HD] for h in heads], axis=1)
        wk = np.concatenate([w_qkv[:, DM + h * HD:DM + (h + 1) * HD] for h in heads], axis=1)
        wvv = np.concatenate([w_qkv[:, 2 * DM + h * HD:2 * DM + (h + 1) * HD] for h in heads], axis=1)
        wqkc = np.concatenate([wq, wk], axis=1).astype(bf)               # [DM, 512]
        wvv = wvv.astype(bf)                                             # [DM, 256]
        wpl = w_proj[g * 256:(g + 1) * 256, :].astype(bf)                # [256, DM]
        in_maps.append({
            "xT": xTc,
            "wqk": np.ascontiguousarray(wqkc),
            "wv": np.ascontiguousarray(wvv),
            "wp": np.ascontiguousarray(wpl),
            "cos_t": cos,
            "sin_t": sin_signed,
        })
    return in_maps


def kernel(x, w_qkv, w_proj, b_proj):
    from concourse.bass_utils import run_bass_kernel_spmd

    if "nc" not in _cache:
        _cache["nc"] = _build_nc()
    nc = _cache["nc"]

    in_maps = _make_in_maps(x, w_qkv, w_proj)
    res = run_bass_kernel_spmd(nc, in_maps, core_ids=list(range(NCORES)))
    outs = [r["out_partial"].astype(np.float32) for r in res.results]
    b_proj = np.asarray(b_proj, dtype=np.float32)
    full = np.empty((B, S, DM), dtype=np.float32)
    for b in range(B):
        full[b] = (outs[4 * b] + outs[4 * b + 1] + outs[4 * b + 2]
                   + outs[4 * b + 3]) + b_proj
    return full
